# revision 11
# baseline (speedup 1.0000x reference)
"""Trainium2 Bass kernel for nn_Decoder_10230612099842.

2-layer decoder (rmsnorm / qkv+RoPE / causal attention / o-proj / rmsnorm /
silu-gated MLP / final rmsnorm) over a ragged-chunk-expanded input.

Strategy (8 NeuronCores = one TRN2 chip):
  - Host: ragged chunk expansion (searchsorted gather) + residual add, weight
    norm-folding, bf16 cast, head-dim pair-permutation for RoPE, per-core
    slicing.
  - Device: data-parallel over batch (2 groups of 4 cores), sequence-parallel
    over tokens within a group (512 tokens/core). Activations live
    feature-major [D, S].  Per layer, one bf16 AllGather of k and one of v
    inside each 4-core group; attention runs a uniform 16-slot loop per head
    with per-core causal mask inputs (mask applied multiplicatively to
    exp-scores; a ones-column appended to v makes masked slots drop out of
    both the softmax numerator and denominator).
  - Matmuls in bf16 with fp32 PSUM accumulation; residual stream fp32.
"""

import numpy as np
import ml_dtypes
from contextlib import ExitStack

import concourse.bass as bass
import concourse.tile as tile
from concourse import bacc, mybir
from concourse.bass_utils import run_bass_kernel_spmd

F32 = mybir.dt.float32
BF16 = mybir.dt.bfloat16
AF = mybir.ActivationFunctionType

# model constants (full problem)
B, K, T, D, NH, HD, DFF, L = 2, 512, 2048, 1024, 16, 64, 4096, 2
EPS = 1e-5
G = 4  # cores per batch group

# stream_shuffle mask: swap adjacent partition pairs within each 32-quadrant
PAIR_MASK = [i ^ 1 for i in range(32)]


def build_decoder(T_, S_, D_, NH_, HD_, DFF_, L_, n_cores=8, sim_safe=False):
    """Build the SPMD decoder graph. S_ = tokens per core, T_ = total tokens
    per batch. Group size G divides cores into batch groups."""
    Dt = D_ // 128          # D partition-tiles
    QT = (NH_ * HD_) // 128  # head-pair tiles (2 heads per tile)
    KT = T_ // 128          # k-slots per head
    TT = S_ // 128          # local token tiles
    CH = max(1, DFF_ // 1024)   # mlp chunks
    DFT = (DFF_ // CH) // 128   # dff tiles per chunk (8)
    WV = min(512, NH_ * HD_)    # v-proj psum width
    VH = (NH_ * HD_) // WV      # v-proj col halves
    HPV = WV // HD_             # heads per v-psum
    scale = 1.0 / float(np.sqrt(HD_))
    n_groups = n_cores // G
    rg = [list(range(g * G, (g + 1) * G)) for g in range(n_groups)]

    nc = bacc.Bacc("TRN2", target_bir_lowering=False, debug=False,
                   num_devices=n_cores)

    # ---- I/O ----
    x0T = nc.dram_tensor("x0T", [D_, S_], F32, kind="ExternalInput")
    wqk = nc.dram_tensor("wqk", [L_, 2 * QT, Dt, 128, 128], BF16,
                         kind="ExternalInput")  # packed q|k lhsT tiles
    wv = nc.dram_tensor("wv", [L_, D_, D_], BF16, kind="ExternalInput")
    wo = nc.dram_tensor("wo", [L_, Dt, QT, 128, 128], BF16,
                        kind="ExternalInput")  # packed per-dt slabs
    w13 = nc.dram_tensor("w13", [L_, CH, DFT, 2, Dt, 128, 128], BF16,
                         kind="ExternalInput")
    w2 = nc.dram_tensor("w2", [L_, CH, Dt, DFT, 128, 128], BF16,
                        kind="ExternalInput")
    cosP = nc.dram_tensor("cosP", [128, S_], BF16, kind="ExternalInput")
    sinP = nc.dram_tensor("sinP", [128, S_], BF16, kind="ExternalInput")
    masks = nc.dram_tensor("masks", [KT, 128, S_], BF16, kind="ExternalInput")
    fw = nc.dram_tensor("fw", [128, Dt], F32, kind="ExternalInput")
    out = nc.dram_tensor("out", [D_, S_], F32, kind="ExternalOutput")

    with tile.TileContext(nc) as tc, ExitStack() as ctx:
        # ---- pools ----
        singles = ctx.enter_context(tc.tile_pool(name="singles", bufs=1))
        wqk_p = ctx.enter_context(tc.tile_pool(name="wqk", bufs=3))
        wv_p = ctx.enter_context(tc.tile_pool(name="wv", bufs=Dt))
        wo_p = ctx.enter_context(tc.tile_pool(name="wo", bufs=3))
        w13_p = ctx.enter_context(tc.tile_pool(name="w13", bufs=3))
        w2_p = ctx.enter_context(tc.tile_pool(name="w2", bufs=3))
        h_p = ctx.enter_context(tc.tile_pool(name="h", bufs=Dt))
        q_p = ctx.enter_context(tc.tile_pool(name="q", bufs=QT))
        vstg_p = ctx.enter_context(tc.tile_pool(name="vstg", bufs=TT))
        kslab_p = ctx.enter_context(tc.tile_pool(name="kslab", bufs=2))
        vslab_p = ctx.enter_context(tc.tile_pool(name="vslab", bufs=3))
        e_p = ctx.enter_context(tc.tile_pool(name="e", bufs=3))
        tmp_p = ctx.enter_context(tc.tile_pool(name="tmp", bufs=2))
        oh_p = ctx.enter_context(tc.tile_pool(name="oh", bufs=QT))
        prod_p = ctx.enter_context(tc.tile_pool(name="prod", bufs=DFT + 1))
        oacc_p = ctx.enter_context(tc.tile_pool(name="oacc", bufs=Dt))
        sq_p = ctx.enter_context(tc.tile_pool(name="sq", bufs=3))
        small_p = ctx.enter_context(tc.tile_pool(name="small", bufs=2))
        ps_mm = ctx.enter_context(tc.tile_pool(name="ps_mm", bufs=4,
                                               space="PSUM"))
        ps_av = ctx.enter_context(tc.tile_pool(name="ps_av", bufs=2,
                                               space="PSUM"))
        ps_ms = ctx.enter_context(tc.tile_pool(name="ps_ms", bufs=2,
                                               space="PSUM"))
        dram = ctx.enter_context(tc.tile_pool(name="dram", bufs=2,
                                              space="DRAM"))

        # ---- persistent constants ----
        ones_col = singles.tile([128, 1], BF16, tag="ones_col")
        nc.vector.memset(ones_col[:], 1.0)
        eps_sb = singles.tile([1, 1], F32, tag="eps")
        nc.vector.memset(eps_sb[:], EPS)
        ones_row = singles.tile([1, 128], BF16, tag="ones_row")
        nc.vector.memset(ones_row[:], 1.0)
        cos_sb = singles.tile([128, S_], BF16, tag="cos")
        nc.sync.dma_start(cos_sb[:], cosP[:])
        sin_sb = singles.tile([128, S_], BF16, tag="sin")
        nc.sync.dma_start(sin_sb[:], sinP[:])
        fw_sb = singles.tile([128, Dt], F32, tag="fw")
        nc.sync.dma_start(fw_sb[:], fw[:])
        mask_sb = []
        for s in range(KT):
            m = singles.tile([128, S_], BF16, tag=f"mask{s}")
            nc.sync.dma_start(m[:], masks[s])
            mask_sb.append(m)
        x_sb = []
        for t in range(Dt):
            xt = singles.tile([128, S_], F32, tag=f"x{t}")
            nc.sync.dma_start(xt[:], x0T[t * 128:(t + 1) * 128, :])
            x_sb.append(xt)

        def rmsnorm_bcast(x_tiles):
            """Return a PSUM tile [128, S] holding rstd broadcast to all
            partitions (1/sqrt(mean(x^2)+eps) per token column)."""
            ssum = ps_ms.tile([128, S_], F32, tag="misc")
            for t in range(Dt):
                sq = sq_p.tile([128, S_], BF16, tag="sq")
                nc.vector.tensor_mul(sq[:], x_tiles[t][:], x_tiles[t][:])
                nc.tensor.matmul(ssum[0:1, :], ones_col[:], sq[:],
                                 start=(t == 0), stop=(t == Dt - 1))
            rstd = small_p.tile([1, S_], F32, tag="rstd")
            nc.scalar.activation(rstd[:], ssum[0:1, :], AF.Sqrt,
                                 bias=eps_sb[:], scale=1.0 / D_)
            nc.vector.reciprocal(rstd[:], rstd[:])
            rstd_bf = small_p.tile([1, S_], BF16, tag="rstd_bf")
            nc.vector.tensor_copy(rstd_bf[:], rstd[:])
            rn = ps_ms.tile([128, S_], F32, tag="misc")
            nc.tensor.matmul(rn[:], ones_row[:], rstd_bf[:],
                             start=True, stop=True)
            return rn

        def rmsnorm_to_h(x_tiles):
            rn = rmsnorm_bcast(x_tiles)
            hs = []
            for t in range(Dt):
                ht = h_p.tile([128, S_], BF16, tag="h")
                nc.vector.tensor_mul(ht[:], x_tiles[t][:], rn[:])
                hs.append(ht)
            return hs

        def rope_from_psum(ps, dst):
            """dst(bf16) = ps*cos + shuffle(ps)*sin  (pair-permuted RoPE)."""
            qb = tmp_p.tile([128, S_], BF16, tag="qb")
            nc.vector.tensor_copy(qb[:], ps[:])
            shuf = tmp_p.tile([128, S_], BF16, tag="shuf")
            nc.vector.stream_shuffle(shuf[:], qb[:], PAIR_MASK)
            qc = tmp_p.tile([128, S_], BF16, tag="qc")
            nc.vector.tensor_mul(qc[:], qb[:], cos_sb[:])
            nc.vector.tensor_mul(shuf[:], shuf[:], sin_sb[:])
            nc.vector.tensor_add(dst[:], qc[:], shuf[:])

        for l in range(L_):
            # ---------------- attention sublayer ----------------
            h = rmsnorm_to_h(x_sb)

            # q tiles (feature-major, rope'd, kept in SBUF)
            q_sb = []
            for j in range(QT):
                wj = wqk_p.tile([128, Dt * 128], BF16, tag="wqk")
                nc.sync.dma_start(wj.rearrange("p (k c) -> p k c", c=128),
                                  wqk[l, j].rearrange("k p c -> p k c"))
                ps = ps_mm.tile([128, S_], F32, tag="mm")
                for kt in range(Dt):
                    nc.tensor.matmul(ps[:], wj[:, kt * 128:(kt + 1) * 128],
                                     h[kt][:], start=(kt == 0),
                                     stop=(kt == Dt - 1))
                qj = q_p.tile([128, S_], BF16, tag="q")
                rope_from_psum(ps, qj)
                q_sb.append(qj)

            # k tiles -> rope -> stage to DRAM for AllGather
            kin_k = dram.tile([NH_, HD_, S_], BF16, tag="kin_k")
            for j in range(QT):
                wj = wqk_p.tile([128, Dt * 128], BF16, tag="wqk")
                nc.sync.dma_start(wj.rearrange("p (k c) -> p k c", c=128),
                                  wqk[l, QT + j].rearrange("k p c -> p k c"))
                ps = ps_mm.tile([128, S_], F32, tag="mm")
                for kt in range(Dt):
                    nc.tensor.matmul(ps[:], wj[:, kt * 128:(kt + 1) * 128],
                                     h[kt][:], start=(kt == 0),
                                     stop=(kt == Dt - 1))
                kj = tmp_p.tile([128, S_], BF16, tag="kj")
                rope_from_psum(ps, kj)
                nc.sync.dma_start(
                    kin_k[2 * j:2 * j + 2].rearrange("h d s -> (h d) s"),
                    kj[:])

            # v tiles (token-major) -> stage [head, tok, 65] with ones col
            wv_sb = []
            for kt in range(Dt):
                wt = wv_p.tile([128, D_], BF16, tag="wv")
                nc.sync.dma_start(wt[:], wv[l, kt * 128:(kt + 1) * 128, :])
                wv_sb.append(wt)
            kin_v = dram.tile([NH_, S_, 65], BF16, tag="kin_v")
            for tt in range(TT):
                vs = vstg_p.tile([128, NH_ * 65], BF16, tag="vstg")
                for half in range(VH):
                    ps = ps_mm.tile([128, WV], F32, tag="mm")
                    for kt in range(Dt):
                        nc.tensor.matmul(
                            ps[:], h[kt][:, tt * 128:(tt + 1) * 128],
                            wv_sb[kt][:, half * WV:(half + 1) * WV],
                            start=(kt == 0), stop=(kt == Dt - 1))
                    for hh in range(HPV):
                        hd0 = half * HPV + hh
                        nc.vector.tensor_copy(
                            vs[:, hd0 * 65:hd0 * 65 + 64],
                            ps[:, hh * HD_:(hh + 1) * HD_])
                for hd0 in range(NH_):
                    nc.vector.memset(vs[:, hd0 * 65 + 64:hd0 * 65 + 65], 1.0)
                nc.sync.dma_start(
                    kin_v.rearrange("h s c -> s h c")[tt * 128:(tt + 1) * 128],
                    vs.rearrange("p (h c) -> p h c", c=65))

            # AllGather k and v within each batch group
            kout_k = dram.tile([G, NH_, HD_, S_], BF16, tag="kout_k")
            kout_v = dram.tile([G, NH_, S_, 65], BF16, tag="kout_v")
            nc.gpsimd.collective_compute(
                "AllGather", mybir.AluOpType.bypass, replica_groups=rg,
                ins=[kin_k.opt()], outs=[kout_k.opt()])
            nc.gpsimd.collective_compute(
                "AllGather", mybir.AluOpType.bypass, replica_groups=rg,
                ins=[kin_v.opt()], outs=[kout_v.opt()])

            # attention per head: uniform KT-slot loop, mask-multiplied exp
            oh_sb = []
            kpair = None
            for head in range(NH_):
                par = head % 2
                if par == 0:
                    kpair = kslab_p.tile([128, KT * 128], BF16, tag="kslab")
                    for hh in range(2):
                        nc.sync.dma_start(
                            kpair[hh * 64:(hh + 1) * 64, :].rearrange(
                                "d (g s) -> d g s", g=G),
                            kout_k[:, head + hh].rearrange("g d s -> d g s"))
                vslab = vslab_p.tile([128, KT, 65], BF16, tag="vslab")
                for g in range(G):
                    nc.sync.dma_start(
                        vslab[:, g * TT:(g + 1) * TT, :],
                        kout_v[g, head].rearrange("(t p) c -> p t c", p=128))
                av = ps_av.tile([65, S_], F32, tag="av")
                qj = q_sb[head // 2][par * 64:par * 64 + 64, :]
                for s in range(KT):
                    sc = ps_mm.tile([128, S_], F32, tag="mm")
                    nc.tensor.matmul(sc[:],
                                     kpair[par * 64:par * 64 + 64,
                                           s * 128:(s + 1) * 128],
                                     qj, start=True, stop=True)
                    e = e_p.tile([128, S_], BF16, tag="e")
                    nc.scalar.activation(e[:], sc[:], AF.Exp, scale=scale)
                    nc.vector.tensor_mul(e[:], e[:], mask_sb[s][:])
                    nc.tensor.matmul(av[:], vslab[:, s, :], e[:],
                                     start=(s == 0), stop=(s == KT - 1))
                recip = small_p.tile([1, S_], F32, tag="recip")
                nc.vector.reciprocal(recip[:], av[64:65, :])
                recip_bf = small_p.tile([1, S_], BF16, tag="recip_bf")
                nc.vector.tensor_copy(recip_bf[:], recip[:])
                rb = ps_ms.tile([128, S_], F32, tag="misc")
                nc.tensor.matmul(rb[0:64, :], ones_row[:, 0:64], recip_bf[:],
                                 start=True, stop=True)
                rb_sb = tmp_p.tile([64, S_], BF16, tag="rb_sb")
                nc.vector.tensor_copy(rb_sb[:], rb[0:64, :])
                if par == 0:
                    ohp = oh_p.tile([128, S_], BF16, tag="oh")
                    oh_sb.append(ohp)
                # odd head writes the upper partition half (64-ch DVE ops may
                # target either half)
                nc.vector.tensor_mul(oh_sb[-1][par * 64:par * 64 + 64, :],
                                     av[0:64, :], rb_sb[:])

            # o-projection + residual (K=128 per head pair)
            for dt in range(Dt):
                wos = wo_p.tile([128, QT * 128], BF16, tag="wo")
                nc.sync.dma_start(wos.rearrange("p (j c) -> p j c", c=128),
                                  wo[l, dt].rearrange("j p c -> p j c"))
                ps = ps_mm.tile([128, S_], F32, tag="mm")
                for jp in range(QT):
                    nc.tensor.matmul(ps[:], wos[:, jp * 128:(jp + 1) * 128],
                                     oh_sb[jp][:], start=(jp == 0),
                                     stop=(jp == QT - 1))
                nc.vector.tensor_add(x_sb[dt][:], x_sb[dt][:], ps[:])

            # ---------------- mlp sublayer ----------------
            h2 = rmsnorm_to_h(x_sb)
            oacc = []
            for ch in range(CH):
                prods = []
                for df in range(DFT):
                    w13s = w13_p.tile([128, 2 * Dt * 128], BF16, tag="w13")
                    nc.sync.dma_start(
                        w13s.rearrange("p (u k c) -> p u k c", u=2, c=128),
                        w13[l, ch, df].rearrange("u k p c -> p u k c"))
                    gps = ps_mm.tile([128, S_], F32, tag="mm")
                    ups = ps_mm.tile([128, S_], F32, tag="mm")
                    for kt in range(Dt):
                        nc.tensor.matmul(gps[:],
                                         w13s[:, kt * 128:(kt + 1) * 128],
                                         h2[kt][:], start=(kt == 0),
                                         stop=(kt == Dt - 1))
                    for kt in range(Dt):
                        off = Dt * 128
                        nc.tensor.matmul(
                            ups[:], w13s[:, off + kt * 128:off + (kt + 1) * 128],
                            h2[kt][:], start=(kt == 0), stop=(kt == Dt - 1))
                    gs = e_p.tile([128, S_], BF16, tag="e")
                    pr = prod_p.tile([128, S_], BF16, tag="prod")
                    if sim_safe:
                        # CoreSim lacks Silu; sigmoid + explicit mul
                        nc.scalar.activation(gs[:], gps[:], AF.Sigmoid)
                        gg = tmp_p.tile([128, S_], BF16, tag="gg")
                        nc.vector.tensor_mul(gg[:], gs[:], gps[:])
                        nc.vector.tensor_mul(pr[:], gg[:], ups[:])
                    else:
                        nc.scalar.activation(gs[:], gps[:], AF.Silu)
                        nc.vector.tensor_mul(pr[:], gs[:], ups[:])
                    prods.append(pr)
                for dt in range(Dt):
                    w2s = w2_p.tile([128, DFT * 128], BF16, tag="w2")
                    nc.sync.dma_start(
                        w2s.rearrange("p (j c) -> p j c", c=128),
                        w2[l, ch, dt].rearrange("j p c -> p j c"))
                    ps = ps_mm.tile([128, S_], F32, tag="mm")
                    for j in range(DFT):
                        nc.tensor.matmul(ps[:], w2s[:, j * 128:(j + 1) * 128],
                                         prods[j][:], start=(j == 0),
                                         stop=(j == DFT - 1))
                    if ch == 0:
                        oa = oacc_p.tile([128, S_], F32, tag="oacc")
                        nc.vector.tensor_copy(oa[:], ps[:])
                        oacc.append(oa)
                    else:
                        nc.vector.tensor_add(oacc[dt][:], oacc[dt][:], ps[:])
            for dt in range(Dt):
                nc.vector.tensor_add(x_sb[dt][:], x_sb[dt][:], oacc[dt][:])

        # ---------------- final rmsnorm ----------------
        rn = rmsnorm_bcast(x_sb)
        for dt in range(Dt):
            xn = tmp_p.tile([128, S_], F32, tag="xn")
            nc.vector.tensor_mul(xn[:], x_sb[dt][:], rn[:])
            nc.vector.tensor_scalar_mul(xn[:], xn[:], fw_sb[:, dt:dt + 1])
            nc.sync.dma_start(out[dt * 128:(dt + 1) * 128, :], xn[:])

    nc.compile()
    return nc


# ---------------------------------------------------------------------------
# host-side preparation
# ---------------------------------------------------------------------------

def _bf16(a):
    return np.ascontiguousarray(np.asarray(a, dtype=np.float32)).astype(
        ml_dtypes.bfloat16)


def _perm(HD_):
    """Head-dim pair permutation: slot 2i <- dim i, slot 2i+1 <- dim i+HD/2."""
    half = HD_ // 2
    p = np.empty(HD_, dtype=np.int64)
    p[0::2] = np.arange(half)
    p[1::2] = np.arange(half) + half
    return p


def prepare_in_maps(x0, cos, sin, wq, wk, wv_, wo_, anw, mnw, w1, w3, w2_,
                    fnw, T_, S_, D_, NH_, HD_, DFF_, L_, n_cores=8):
    """Build per-core input dicts. x0 is the already-expanded [B', T, D] fp32
    input (B' = n_cores // G batches)."""
    Dt = D_ // 128
    QT = (NH_ * HD_) // 128
    KT = T_ // 128
    CH = max(1, DFF_ // 1024)
    DFT = (DFF_ // CH) // 128
    perm = _perm(HD_)
    half = HD_ // 2

    # fold norm weights into the consuming projections
    wq_e = anw[:, :, None] * wq      # [L, D, D]
    wk_e = anw[:, :, None] * wk
    wv_e = anw[:, :, None] * wv_
    w1_e = mnw[:, :, None] * w1      # [L, D, DFF]
    w3_e = mnw[:, :, None] * w3

    # permute q/k columns per head by `perm`
    def permute_cols(w):
        wh = w.reshape(L_, D_, NH_, HD_)
        return wh[:, :, :, perm].reshape(L_, D_, NH_ * HD_)

    wq_p = permute_cols(wq_e)
    wk_p = permute_cols(wk_e)

    # packed q|k lhsT tiles: [L, 2QT, Dt, 128, 128]
    wqk_pack = np.empty((L_, 2 * QT, Dt, 128, 128), dtype=np.float32)
    for j in range(QT):
        for kt in range(Dt):
            wqk_pack[:, j, kt] = wq_p[:, kt * 128:(kt + 1) * 128,
                                      j * 128:(j + 1) * 128]
            wqk_pack[:, QT + j, kt] = wk_p[:, kt * 128:(kt + 1) * 128,
                                           j * 128:(j + 1) * 128]
    # wo packed: [L, Dt, QT, 128, 128]; rows = o dims (head-major)
    wo_pack = np.empty((L_, Dt, QT, 128, 128), dtype=np.float32)
    for dt in range(Dt):
        for j in range(QT):
            wo_pack[:, dt, j] = wo_[:, j * 128:(j + 1) * 128,
                                    dt * 128:(dt + 1) * 128]
    # w13 packed: [L, CH, DFT, 2, Dt, 128, 128]
    csz = DFF_ // CH
    w13_pack = np.empty((L_, CH, DFT, 2, Dt, 128, 128), dtype=np.float32)
    for ch in range(CH):
        for df in range(DFT):
            c0 = ch * csz + df * 128
            for kt in range(Dt):
                w13_pack[:, ch, df, 0, kt] = w1_e[:, kt * 128:(kt + 1) * 128,
                                                  c0:c0 + 128]
                w13_pack[:, ch, df, 1, kt] = w3_e[:, kt * 128:(kt + 1) * 128,
                                                  c0:c0 + 128]
    # w2 packed: [L, CH, Dt, DFT, 128, 128]
    w2_pack = np.empty((L_, CH, Dt, DFT, 128, 128), dtype=np.float32)
    for ch in range(CH):
        for dt in range(Dt):
            for j in range(DFT):
                r0 = ch * csz + j * 128
                w2_pack[:, ch, dt, j] = w2_[:, r0:r0 + 128,
                                            dt * 128:(dt + 1) * 128]

    wqk_b = _bf16(wqk_pack)
    wv_b = _bf16(wv_e)
    wo_b = _bf16(wo_pack)
    w13_b = _bf16(w13_pack)
    w2_b = _bf16(w2_pack)
    fw_np = np.ascontiguousarray(
        np.asarray(fnw, np.float32).reshape(Dt, 128).T)  # [128, Dt]

    # rope tables, permuted + sign-baked, duplicated per head pair -> [128, T]
    cosPf = np.asarray(cos, np.float32)[:, perm].T        # [HD, T]
    sinf = np.asarray(sin, np.float32)[:, perm].T         # [HD, T]
    sign = np.where(np.arange(HD_) % 2 == 0, -1.0, 1.0)[:, None]
    sinPf = sinf * sign
    cosP2 = np.tile(cosPf, (2, 1))                        # [128, T]
    sinP2 = np.tile(sinPf, (2, 1))

    in_maps = []
    for c in range(n_cores):
        b = c // G
        r = c % G
        t0 = r * S_
        xs = np.ascontiguousarray(x0[b, t0:t0 + S_, :].T).astype(np.float32)
        mask = np.zeros((KT, 128, S_), dtype=np.float32)
        for s in range(KT):
            kg = 128 * s + np.arange(128)[:, None]
            qg = t0 + np.arange(S_)[None, :]
            mask[s] = (kg <= qg).astype(np.float32)
        in_maps.append({
            "x0T": xs,
            "wqk": wqk_b, "wv": wv_b, "wo": wo_b, "w13": w13_b, "w2": w2_b,
            "cosP": _bf16(cosP2[:, t0:t0 + S_]),
            "sinP": _bf16(sinP2[:, t0:t0 + S_]),
            "masks": mask.astype(ml_dtypes.bfloat16),
            "fw": fw_np,
        })
    return in_maps


def expand_input(x_processed, boundaries, counts, x_residual):
    """Ragged chunk expansion: token t of batch b takes chunk
    #{boundaries[b] <= t}, plus residual."""
    xp = np.asarray(x_processed, np.float32)
    bd = np.asarray(boundaries)
    xr = np.asarray(x_residual, np.float32)
    Bn, Tn, Dn = xr.shape
    tt = np.arange(Tn)
    out = np.empty_like(xr)
    for b in range(Bn):
        idx = np.searchsorted(bd[b], tt, side="right")
        out[b] = xp[b, idx, :] + xr[b]
    return out


_NC_CACHE = {}


def _get_nc(key):
    if key not in _NC_CACHE:
        _NC_CACHE[key] = build_decoder(*key)
    return _NC_CACHE[key]


def kernel(x_processed, boundaries, counts, x_residual, cos, sin, seq_len,
           wq, wk, wv, wo, attn_norm_w, mlp_norm_w, w1, w3, w2, final_norm_w,
           _trace=False):
    S_ = T // G
    x0 = expand_input(x_processed, boundaries, counts, x_residual)
    in_maps = prepare_in_maps(
        x0, cos, sin,
        np.asarray(wq, np.float32), np.asarray(wk, np.float32),
        np.asarray(wv, np.float32), np.asarray(wo, np.float32),
        np.asarray(attn_norm_w, np.float32), np.asarray(mlp_norm_w, np.float32),
        np.asarray(w1, np.float32), np.asarray(w3, np.float32),
        np.asarray(w2, np.float32), np.asarray(final_norm_w, np.float32),
        T, S_, D, NH, HD, DFF, L, n_cores=8)
    nc = _get_nc((T, S_, D, NH, HD, DFF, L, 8))
    res = run_bass_kernel_spmd(nc, in_maps, list(range(8)), trace=_trace)
    outp = np.empty((B, T, D), dtype=np.float32)
    for c in range(8):
        b, r = c // G, c % G
        outp[b, r * S_:(r + 1) * S_, :] = res.results[c]["out"].T
    if _trace:
        kernel.last_exec_time_ns = res.exec_time_ns
        kernel.last_results = res
    return outp


# revision 14
# speedup vs baseline: 1.0005x; 1.0005x over previous
"""Trainium2 Bass kernel for nn_Decoder_10230612099842.

2-layer decoder (rmsnorm / qkv+RoPE / causal attention / o-proj / rmsnorm /
silu-gated MLP / final rmsnorm) over a ragged-chunk-expanded input.

Strategy (8 NeuronCores = one TRN2 chip):
  - Host: ragged chunk expansion (searchsorted gather) + residual add, weight
    norm-folding, bf16 cast, head-dim pair-permutation for RoPE, per-core
    slicing.
  - Device: data-parallel over batch (2 groups of 4 cores), sequence-parallel
    over tokens within a group (512 tokens/core). Activations live
    feature-major [D, S].  Per layer, one bf16 AllGather of k and one of v
    inside each 4-core group; attention runs a uniform 16-slot loop per head
    with per-core causal mask inputs (mask applied multiplicatively to
    exp-scores; a ones-column appended to v makes masked slots drop out of
    both the softmax numerator and denominator).
  - Matmuls in bf16 with fp32 PSUM accumulation; residual stream fp32.
"""

import numpy as np
import ml_dtypes
from contextlib import ExitStack

import concourse.bass as bass
import concourse.tile as tile
from concourse import bacc, mybir
from concourse.bass_utils import run_bass_kernel_spmd

F32 = mybir.dt.float32
BF16 = mybir.dt.bfloat16
AF = mybir.ActivationFunctionType

# model constants (full problem)
B, K, T, D, NH, HD, DFF, L = 2, 512, 2048, 1024, 16, 64, 4096, 2
EPS = 1e-5
G = 4  # cores per batch group

# stream_shuffle mask: swap adjacent partition pairs within each 32-quadrant
PAIR_MASK = [i ^ 1 for i in range(32)]


def build_decoder(T_, S_, D_, NH_, HD_, DFF_, L_, n_cores=8, sim_safe=False):
    """Build the SPMD decoder graph. S_ = tokens per core, T_ = total tokens
    per batch. Group size G divides cores into batch groups."""
    Dt = D_ // 128          # D partition-tiles
    QT = (NH_ * HD_) // 128  # head-pair tiles (2 heads per tile)
    KT = T_ // 128          # k-slots per head
    TT = S_ // 128          # local token tiles
    CH = max(1, DFF_ // 1024)   # mlp chunks
    DFT = (DFF_ // CH) // 128   # dff tiles per chunk (8)
    WV = min(512, NH_ * HD_)    # v-proj psum width
    VH = (NH_ * HD_) // WV      # v-proj col halves
    HPV = WV // HD_             # heads per v-psum
    scale = 1.0 / float(np.sqrt(HD_))
    n_groups = n_cores // G
    rg = [list(range(g * G, (g + 1) * G)) for g in range(n_groups)]

    nc = bacc.Bacc("TRN2", target_bir_lowering=False, debug=False,
                   num_devices=n_cores)

    # ---- I/O ----
    x0T = nc.dram_tensor("x0T", [D_, S_], F32, kind="ExternalInput")
    wqk = nc.dram_tensor("wqk", [L_, 2 * QT, Dt, 128, 128], BF16,
                         kind="ExternalInput")  # packed q|k lhsT tiles
    wv = nc.dram_tensor("wv", [L_, D_, D_], BF16, kind="ExternalInput")
    wo = nc.dram_tensor("wo", [L_, Dt, QT, 128, 128], BF16,
                        kind="ExternalInput")  # packed per-dt slabs
    w13 = nc.dram_tensor("w13", [L_, CH, DFT, 2, Dt, 128, 128], BF16,
                         kind="ExternalInput")
    w2 = nc.dram_tensor("w2", [L_, CH, Dt, DFT, 128, 128], BF16,
                        kind="ExternalInput")
    cosP = nc.dram_tensor("cosP", [128, S_], BF16, kind="ExternalInput")
    sinP = nc.dram_tensor("sinP", [128, S_], BF16, kind="ExternalInput")
    masks = nc.dram_tensor("masks", [KT, 128, S_], BF16, kind="ExternalInput")
    fw = nc.dram_tensor("fw", [128, Dt], F32, kind="ExternalInput")
    out = nc.dram_tensor("out", [D_, S_], F32, kind="ExternalOutput")

    with tile.TileContext(nc) as tc, ExitStack() as ctx:
        # ---- pools ----
        singles = ctx.enter_context(tc.tile_pool(name="singles", bufs=1))
        wqk_p = ctx.enter_context(tc.tile_pool(name="wqk", bufs=3))
        wv_p = ctx.enter_context(tc.tile_pool(name="wv", bufs=Dt))
        wo_p = ctx.enter_context(tc.tile_pool(name="wo", bufs=3))
        w13_p = ctx.enter_context(tc.tile_pool(name="w13", bufs=3))
        w2_p = ctx.enter_context(tc.tile_pool(name="w2", bufs=3))
        h_p = ctx.enter_context(tc.tile_pool(name="h", bufs=Dt))
        q_p = ctx.enter_context(tc.tile_pool(name="q", bufs=QT))
        vstg_p = ctx.enter_context(tc.tile_pool(name="vstg", bufs=TT))
        kslab_p = ctx.enter_context(tc.tile_pool(name="kslab", bufs=2))
        vslab_p = ctx.enter_context(tc.tile_pool(name="vslab", bufs=3))
        e_p = ctx.enter_context(tc.tile_pool(name="e", bufs=4))
        tmp_p = ctx.enter_context(tc.tile_pool(name="tmp", bufs=2))
        oh_p = ctx.enter_context(tc.tile_pool(name="oh", bufs=QT))
        prod_p = ctx.enter_context(tc.tile_pool(name="prod", bufs=DFT + 1))
        oacc_p = ctx.enter_context(tc.tile_pool(name="oacc", bufs=Dt))
        sq_p = ctx.enter_context(tc.tile_pool(name="sq", bufs=3))
        small_p = ctx.enter_context(tc.tile_pool(name="small", bufs=2))
        ps_mm = ctx.enter_context(tc.tile_pool(name="ps_mm", bufs=4,
                                               space="PSUM"))
        ps_av = ctx.enter_context(tc.tile_pool(name="ps_av", bufs=2,
                                               space="PSUM"))
        ps_ms = ctx.enter_context(tc.tile_pool(name="ps_ms", bufs=2,
                                               space="PSUM"))
        dram = ctx.enter_context(tc.tile_pool(name="dram", bufs=2,
                                              space="DRAM"))

        # ---- persistent constants ----
        ones_col = singles.tile([128, 1], BF16, tag="ones_col")
        nc.vector.memset(ones_col[:], 1.0)
        eps_sb = singles.tile([1, 1], F32, tag="eps")
        nc.vector.memset(eps_sb[:], EPS)
        ones_row = singles.tile([1, 128], BF16, tag="ones_row")
        nc.vector.memset(ones_row[:], 1.0)
        cos_sb = singles.tile([128, S_], BF16, tag="cos")
        nc.sync.dma_start(cos_sb[:], cosP[:])
        sin_sb = singles.tile([128, S_], BF16, tag="sin")
        nc.sync.dma_start(sin_sb[:], sinP[:])
        fw_sb = singles.tile([128, Dt], F32, tag="fw")
        nc.sync.dma_start(fw_sb[:], fw[:])
        mask_sb = []
        for s in range(KT):
            m = singles.tile([128, S_], BF16, tag=f"mask{s}")
            nc.sync.dma_start(m[:], masks[s])
            mask_sb.append(m)
        x_sb = []
        for t in range(Dt):
            xt = singles.tile([128, S_], F32, tag=f"x{t}")
            nc.sync.dma_start(xt[:], x0T[t * 128:(t + 1) * 128, :])
            x_sb.append(xt)

        def rmsnorm_bcast(x_tiles):
            """Return a PSUM tile [128, S] holding rstd broadcast to all
            partitions (1/sqrt(mean(x^2)+eps) per token column)."""
            ssum = ps_ms.tile([128, S_], F32, tag="misc")
            for t in range(Dt):
                sq = sq_p.tile([128, S_], BF16, tag="sq")
                nc.vector.tensor_mul(sq[:], x_tiles[t][:], x_tiles[t][:])
                nc.tensor.matmul(ssum[0:1, :], ones_col[:], sq[:],
                                 start=(t == 0), stop=(t == Dt - 1))
            rstd = small_p.tile([1, S_], F32, tag="rstd")
            nc.scalar.activation(rstd[:], ssum[0:1, :], AF.Sqrt,
                                 bias=eps_sb[:], scale=1.0 / D_)
            nc.vector.reciprocal(rstd[:], rstd[:])
            rstd_bf = small_p.tile([1, S_], BF16, tag="rstd_bf")
            nc.vector.tensor_copy(rstd_bf[:], rstd[:])
            rn = ps_ms.tile([128, S_], F32, tag="misc")
            nc.tensor.matmul(rn[:], ones_row[:], rstd_bf[:],
                             start=True, stop=True)
            return rn

        def rmsnorm_to_h(x_tiles):
            rn = rmsnorm_bcast(x_tiles)
            hs = []
            for t in range(Dt):
                ht = h_p.tile([128, S_], BF16, tag="h")
                nc.vector.tensor_mul(ht[:], x_tiles[t][:], rn[:])
                hs.append(ht)
            return hs

        def rope_from_psum(ps, dst):
            """dst(bf16) = ps*cos + shuffle(ps)*sin  (pair-permuted RoPE)."""
            qb = tmp_p.tile([128, S_], BF16, tag="qb")
            nc.vector.tensor_copy(qb[:], ps[:])
            shuf = tmp_p.tile([128, S_], BF16, tag="shuf")
            nc.vector.stream_shuffle(shuf[:], qb[:], PAIR_MASK)
            qc = tmp_p.tile([128, S_], BF16, tag="qc")
            nc.vector.tensor_mul(qc[:], qb[:], cos_sb[:])
            nc.vector.tensor_mul(shuf[:], shuf[:], sin_sb[:])
            nc.vector.tensor_add(dst[:], qc[:], shuf[:])

        for l in range(L_):
            # ---------------- attention sublayer ----------------
            h = rmsnorm_to_h(x_sb)

            # k tiles first -> rope -> stage -> kick AllGather early
            kin_k = dram.tile([NH_, HD_, S_], BF16, tag="kin_k")
            for j in range(QT):
                wj = wqk_p.tile([128, Dt * 128], BF16, tag="wqk")
                nc.sync.dma_start(wj.rearrange("p (k c) -> p k c", c=128),
                                  wqk[l, QT + j].rearrange("k p c -> p k c"))
                ps = ps_mm.tile([128, S_], F32, tag="mm")
                for kt in range(Dt):
                    nc.tensor.matmul(ps[:], wj[:, kt * 128:(kt + 1) * 128],
                                     h[kt][:], start=(kt == 0),
                                     stop=(kt == Dt - 1))
                kj = tmp_p.tile([128, S_], BF16, tag="kj")
                rope_from_psum(ps, kj)
                nc.sync.dma_start(
                    kin_k[2 * j:2 * j + 2].rearrange("h d s -> (h d) s"),
                    kj[:])
            kout_k = dram.tile([G, NH_, HD_, S_], BF16, tag="kout_k")
            nc.gpsimd.collective_compute(
                "AllGather", mybir.AluOpType.bypass, replica_groups=rg,
                ins=[kin_k.opt()], outs=[kout_k.opt()])

            # v tiles (token-major) -> stage [head, tok, 65] -> AllGather
            wv_sb = []
            for kt in range(Dt):
                wt = wv_p.tile([128, D_], BF16, tag="wv")
                nc.sync.dma_start(wt[:], wv[l, kt * 128:(kt + 1) * 128, :])
                wv_sb.append(wt)
            kin_v = dram.tile([NH_, S_, 65], BF16, tag="kin_v")
            for tt in range(TT):
                vs = vstg_p.tile([128, NH_ * 65], BF16, tag="vstg")
                for half in range(VH):
                    ps = ps_mm.tile([128, WV], F32, tag="mm")
                    for kt in range(Dt):
                        nc.tensor.matmul(
                            ps[:], h[kt][:, tt * 128:(tt + 1) * 128],
                            wv_sb[kt][:, half * WV:(half + 1) * WV],
                            start=(kt == 0), stop=(kt == Dt - 1))
                    for hh in range(HPV):
                        hd0 = half * HPV + hh
                        nc.vector.tensor_copy(
                            vs[:, hd0 * 65:hd0 * 65 + 64],
                            ps[:, hh * HD_:(hh + 1) * HD_])
                for hd0 in range(NH_):
                    nc.vector.memset(vs[:, hd0 * 65 + 64:hd0 * 65 + 65], 1.0)
                nc.sync.dma_start(
                    kin_v.rearrange("h s c -> s h c")[tt * 128:(tt + 1) * 128],
                    vs.rearrange("p (h c) -> p h c", c=65))
            kout_v = dram.tile([G, NH_, S_, 65], BF16, tag="kout_v")
            nc.gpsimd.collective_compute(
                "AllGather", mybir.AluOpType.bypass, replica_groups=rg,
                ins=[kin_v.opt()], outs=[kout_v.opt()])

            # q tiles (feature-major, rope'd) — overlaps the AllGathers
            q_sb = []
            for j in range(QT):
                wj = wqk_p.tile([128, Dt * 128], BF16, tag="wqk")
                nc.sync.dma_start(wj.rearrange("p (k c) -> p k c", c=128),
                                  wqk[l, j].rearrange("k p c -> p k c"))
                ps = ps_mm.tile([128, S_], F32, tag="mm")
                for kt in range(Dt):
                    nc.tensor.matmul(ps[:], wj[:, kt * 128:(kt + 1) * 128],
                                     h[kt][:], start=(kt == 0),
                                     stop=(kt == Dt - 1))
                qj = q_p.tile([128, S_], BF16, tag="q")
                rope_from_psum(ps, qj)
                q_sb.append(qj)

            # attention, one head PAIR at a time: the two K=64 score matmuls
            # target distinct PE row groups (tile_position auto-derived from
            # the lhsT partition base) and run concurrently in the array
            oh_sb = []
            for j in range(QT):
                kpair = kslab_p.tile([128, KT * 128], BF16, tag="kslab")
                for hh in range(2):
                    nc.sync.dma_start(
                        kpair[hh * 64:(hh + 1) * 64, :].rearrange(
                            "d (g s) -> d g s", g=G),
                        kout_k[:, 2 * j + hh].rearrange("g d s -> d g s"))
                vsl = []
                for hh in range(2):
                    vslab = vslab_p.tile([128, KT, 65], BF16, tag="vslab")
                    for g in range(G):
                        nc.sync.dma_start(
                            vslab[:, g * TT:(g + 1) * TT, :],
                            kout_v[g, 2 * j + hh].rearrange(
                                "(t p) c -> p t c", p=128))
                    vsl.append(vslab)
                av = [ps_av.tile([65, S_], F32, tag="av", name=f"av{j}_{hh2}")
                      for hh2 in range(2)]
                for s in range(KT):
                    es = []
                    for hh in range(2):
                        sc = ps_mm.tile([128, S_], F32, tag="mm")
                        nc.tensor.matmul(sc[:],
                                         kpair[hh * 64:hh * 64 + 64,
                                               s * 128:(s + 1) * 128],
                                         q_sb[j][hh * 64:hh * 64 + 64, :],
                                         start=True, stop=True)
                        e = e_p.tile([128, S_], BF16, tag="e")
                        nc.scalar.activation(e[:], sc[:], AF.Exp, scale=scale)
                        nc.vector.tensor_mul(e[:], e[:], mask_sb[s][:])
                        es.append(e)
                    for hh in range(2):
                        nc.tensor.matmul(av[hh][:], vsl[hh][:, s, :], es[hh][:],
                                         start=(s == 0), stop=(s == KT - 1))
                ohp = oh_p.tile([128, S_], BF16, tag="oh")
                oh_sb.append(ohp)
                for hh in range(2):
                    recip = small_p.tile([1, S_], F32, tag="recip")
                    nc.vector.reciprocal(recip[:], av[hh][64:65, :])
                    recip_bf = small_p.tile([1, S_], BF16, tag="recip_bf")
                    nc.vector.tensor_copy(recip_bf[:], recip[:])
                    rb = ps_ms.tile([128, S_], F32, tag="misc")
                    nc.tensor.matmul(rb[0:64, :], ones_row[:, 0:64],
                                     recip_bf[:], start=True, stop=True)
                    rb_sb = tmp_p.tile([64, S_], BF16, tag="rb_sb")
                    nc.vector.tensor_copy(rb_sb[:], rb[0:64, :])
                    # odd head writes the upper partition half (64-ch DVE ops
                    # may target either half)
                    nc.vector.tensor_mul(ohp[hh * 64:hh * 64 + 64, :],
                                         av[hh][0:64, :], rb_sb[:])

            # o-projection + residual (K=128 per head pair)
            for dt in range(Dt):
                wos = wo_p.tile([128, QT * 128], BF16, tag="wo")
                nc.sync.dma_start(wos.rearrange("p (j c) -> p j c", c=128),
                                  wo[l, dt].rearrange("j p c -> p j c"))
                ps = ps_mm.tile([128, S_], F32, tag="mm")
                for jp in range(QT):
                    nc.tensor.matmul(ps[:], wos[:, jp * 128:(jp + 1) * 128],
                                     oh_sb[jp][:], start=(jp == 0),
                                     stop=(jp == QT - 1))
                nc.vector.tensor_add(x_sb[dt][:], x_sb[dt][:], ps[:])

            # ---------------- mlp sublayer ----------------
            h2 = rmsnorm_to_h(x_sb)
            oacc = []
            for ch in range(CH):
                prods = []
                for df in range(DFT):
                    w13s = w13_p.tile([128, 2 * Dt * 128], BF16, tag="w13")
                    nc.sync.dma_start(
                        w13s.rearrange("p (u k c) -> p u k c", u=2, c=128),
                        w13[l, ch, df].rearrange("u k p c -> p u k c"))
                    gps = ps_mm.tile([128, S_], F32, tag="mm")
                    ups = ps_mm.tile([128, S_], F32, tag="mm")
                    for kt in range(Dt):
                        nc.tensor.matmul(gps[:],
                                         w13s[:, kt * 128:(kt + 1) * 128],
                                         h2[kt][:], start=(kt == 0),
                                         stop=(kt == Dt - 1))
                    for kt in range(Dt):
                        off = Dt * 128
                        nc.tensor.matmul(
                            ups[:], w13s[:, off + kt * 128:off + (kt + 1) * 128],
                            h2[kt][:], start=(kt == 0), stop=(kt == Dt - 1))
                    gs = e_p.tile([128, S_], BF16, tag="e")
                    pr = prod_p.tile([128, S_], BF16, tag="prod")
                    if sim_safe:
                        # CoreSim lacks Silu; sigmoid + explicit mul
                        nc.scalar.activation(gs[:], gps[:], AF.Sigmoid)
                        gg = tmp_p.tile([128, S_], BF16, tag="gg")
                        nc.vector.tensor_mul(gg[:], gs[:], gps[:])
                        nc.vector.tensor_mul(pr[:], gg[:], ups[:])
                    else:
                        nc.scalar.activation(gs[:], gps[:], AF.Silu)
                        nc.vector.tensor_mul(pr[:], gs[:], ups[:])
                    prods.append(pr)
                for dt in range(Dt):
                    w2s = w2_p.tile([128, DFT * 128], BF16, tag="w2")
                    nc.sync.dma_start(
                        w2s.rearrange("p (j c) -> p j c", c=128),
                        w2[l, ch, dt].rearrange("j p c -> p j c"))
                    ps = ps_mm.tile([128, S_], F32, tag="mm")
                    for j in range(DFT):
                        nc.tensor.matmul(ps[:], w2s[:, j * 128:(j + 1) * 128],
                                         prods[j][:], start=(j == 0),
                                         stop=(j == DFT - 1))
                    if ch == 0:
                        oa = oacc_p.tile([128, S_], F32, tag="oacc")
                        nc.vector.tensor_copy(oa[:], ps[:])
                        oacc.append(oa)
                    else:
                        nc.vector.tensor_add(oacc[dt][:], oacc[dt][:], ps[:])
            for dt in range(Dt):
                nc.vector.tensor_add(x_sb[dt][:], x_sb[dt][:], oacc[dt][:])

        # ---------------- final rmsnorm ----------------
        rn = rmsnorm_bcast(x_sb)
        for dt in range(Dt):
            xn = tmp_p.tile([128, S_], F32, tag="xn")
            nc.vector.tensor_mul(xn[:], x_sb[dt][:], rn[:])
            nc.vector.tensor_scalar_mul(xn[:], xn[:], fw_sb[:, dt:dt + 1])
            nc.sync.dma_start(out[dt * 128:(dt + 1) * 128, :], xn[:])

    nc.compile()
    return nc


# ---------------------------------------------------------------------------
# host-side preparation
# ---------------------------------------------------------------------------

def _bf16(a):
    return np.ascontiguousarray(np.asarray(a, dtype=np.float32)).astype(
        ml_dtypes.bfloat16)


def _perm(HD_):
    """Head-dim pair permutation: slot 2i <- dim i, slot 2i+1 <- dim i+HD/2."""
    half = HD_ // 2
    p = np.empty(HD_, dtype=np.int64)
    p[0::2] = np.arange(half)
    p[1::2] = np.arange(half) + half
    return p


def prepare_in_maps(x0, cos, sin, wq, wk, wv_, wo_, anw, mnw, w1, w3, w2_,
                    fnw, T_, S_, D_, NH_, HD_, DFF_, L_, n_cores=8):
    """Build per-core input dicts. x0 is the already-expanded [B', T, D] fp32
    input (B' = n_cores // G batches)."""
    Dt = D_ // 128
    QT = (NH_ * HD_) // 128
    KT = T_ // 128
    CH = max(1, DFF_ // 1024)
    DFT = (DFF_ // CH) // 128
    perm = _perm(HD_)
    half = HD_ // 2

    # fold norm weights into the consuming projections
    wq_e = anw[:, :, None] * wq      # [L, D, D]
    wk_e = anw[:, :, None] * wk
    wv_e = anw[:, :, None] * wv_
    w1_e = mnw[:, :, None] * w1      # [L, D, DFF]
    w3_e = mnw[:, :, None] * w3

    # permute q/k columns per head by `perm`
    def permute_cols(w):
        wh = w.reshape(L_, D_, NH_, HD_)
        return wh[:, :, :, perm].reshape(L_, D_, NH_ * HD_)

    wq_p = permute_cols(wq_e)
    wk_p = permute_cols(wk_e)

    # packed q|k lhsT tiles: [L, 2QT, Dt, 128, 128]
    wqk_pack = np.empty((L_, 2 * QT, Dt, 128, 128), dtype=np.float32)
    for j in range(QT):
        for kt in range(Dt):
            wqk_pack[:, j, kt] = wq_p[:, kt * 128:(kt + 1) * 128,
                                      j * 128:(j + 1) * 128]
            wqk_pack[:, QT + j, kt] = wk_p[:, kt * 128:(kt + 1) * 128,
                                           j * 128:(j + 1) * 128]
    # wo packed: [L, Dt, QT, 128, 128]; rows = o dims (head-major)
    wo_pack = np.empty((L_, Dt, QT, 128, 128), dtype=np.float32)
    for dt in range(Dt):
        for j in range(QT):
            wo_pack[:, dt, j] = wo_[:, j * 128:(j + 1) * 128,
                                    dt * 128:(dt + 1) * 128]
    # w13 packed: [L, CH, DFT, 2, Dt, 128, 128]
    csz = DFF_ // CH
    w13_pack = np.empty((L_, CH, DFT, 2, Dt, 128, 128), dtype=np.float32)
    for ch in range(CH):
        for df in range(DFT):
            c0 = ch * csz + df * 128
            for kt in range(Dt):
                w13_pack[:, ch, df, 0, kt] = w1_e[:, kt * 128:(kt + 1) * 128,
                                                  c0:c0 + 128]
                w13_pack[:, ch, df, 1, kt] = w3_e[:, kt * 128:(kt + 1) * 128,
                                                  c0:c0 + 128]
    # w2 packed: [L, CH, Dt, DFT, 128, 128]
    w2_pack = np.empty((L_, CH, Dt, DFT, 128, 128), dtype=np.float32)
    for ch in range(CH):
        for dt in range(Dt):
            for j in range(DFT):
                r0 = ch * csz + j * 128
                w2_pack[:, ch, dt, j] = w2_[:, r0:r0 + 128,
                                            dt * 128:(dt + 1) * 128]

    wqk_b = _bf16(wqk_pack)
    wv_b = _bf16(wv_e)
    wo_b = _bf16(wo_pack)
    w13_b = _bf16(w13_pack)
    w2_b = _bf16(w2_pack)
    fw_np = np.ascontiguousarray(
        np.asarray(fnw, np.float32).reshape(Dt, 128).T)  # [128, Dt]

    # rope tables, permuted + sign-baked, duplicated per head pair -> [128, T]
    cosPf = np.asarray(cos, np.float32)[:, perm].T        # [HD, T]
    sinf = np.asarray(sin, np.float32)[:, perm].T         # [HD, T]
    sign = np.where(np.arange(HD_) % 2 == 0, -1.0, 1.0)[:, None]
    sinPf = sinf * sign
    cosP2 = np.tile(cosPf, (2, 1))                        # [128, T]
    sinP2 = np.tile(sinPf, (2, 1))

    in_maps = []
    for c in range(n_cores):
        b = c // G
        r = c % G
        t0 = r * S_
        xs = np.ascontiguousarray(x0[b, t0:t0 + S_, :].T).astype(np.float32)
        mask = np.zeros((KT, 128, S_), dtype=np.float32)
        for s in range(KT):
            kg = 128 * s + np.arange(128)[:, None]
            qg = t0 + np.arange(S_)[None, :]
            mask[s] = (kg <= qg).astype(np.float32)
        in_maps.append({
            "x0T": xs,
            "wqk": wqk_b, "wv": wv_b, "wo": wo_b, "w13": w13_b, "w2": w2_b,
            "cosP": _bf16(cosP2[:, t0:t0 + S_]),
            "sinP": _bf16(sinP2[:, t0:t0 + S_]),
            "masks": mask.astype(ml_dtypes.bfloat16),
            "fw": fw_np,
        })
    return in_maps


def expand_input(x_processed, boundaries, counts, x_residual):
    """Ragged chunk expansion: token t of batch b takes chunk
    #{boundaries[b] <= t}, plus residual."""
    xp = np.asarray(x_processed, np.float32)
    bd = np.asarray(boundaries)
    xr = np.asarray(x_residual, np.float32)
    Bn, Tn, Dn = xr.shape
    tt = np.arange(Tn)
    out = np.empty_like(xr)
    for b in range(Bn):
        idx = np.searchsorted(bd[b], tt, side="right")
        out[b] = xp[b, idx, :] + xr[b]
    return out


_NC_CACHE = {}


def _get_nc(key):
    if key not in _NC_CACHE:
        _NC_CACHE[key] = build_decoder(*key)
    return _NC_CACHE[key]


def kernel(x_processed, boundaries, counts, x_residual, cos, sin, seq_len,
           wq, wk, wv, wo, attn_norm_w, mlp_norm_w, w1, w3, w2, final_norm_w,
           _trace=False):
    S_ = T // G
    x0 = expand_input(x_processed, boundaries, counts, x_residual)
    in_maps = prepare_in_maps(
        x0, cos, sin,
        np.asarray(wq, np.float32), np.asarray(wk, np.float32),
        np.asarray(wv, np.float32), np.asarray(wo, np.float32),
        np.asarray(attn_norm_w, np.float32), np.asarray(mlp_norm_w, np.float32),
        np.asarray(w1, np.float32), np.asarray(w3, np.float32),
        np.asarray(w2, np.float32), np.asarray(final_norm_w, np.float32),
        T, S_, D, NH, HD, DFF, L, n_cores=8)
    nc = _get_nc((T, S_, D, NH, HD, DFF, L, 8))
    res = run_bass_kernel_spmd(nc, in_maps, list(range(8)), trace=_trace)
    outp = np.empty((B, T, D), dtype=np.float32)
    for c in range(8):
        b, r = c // G, c % G
        outp[b, r * S_:(r + 1) * S_, :] = res.results[c]["out"].T
    if _trace:
        kernel.last_exec_time_ns = res.exec_time_ns
        kernel.last_results = res
    return outp


# revision 16
# speedup vs baseline: 1.0093x; 1.0088x over previous
"""Trainium2 Bass kernel for nn_Decoder_10230612099842.

2-layer decoder (rmsnorm / qkv+RoPE / causal attention / o-proj / rmsnorm /
silu-gated MLP / final rmsnorm) over a ragged-chunk-expanded input.

Strategy (8 NeuronCores = one TRN2 chip):
  - Host: ragged chunk expansion (searchsorted gather) + residual add, weight
    norm-folding, bf16 cast, head-dim pair-permutation for RoPE, per-core
    slicing.
  - Device: data-parallel over batch (2 groups of 4 cores), sequence-parallel
    over tokens within a group (512 tokens/core). Activations live
    feature-major [D, S].  Per layer, one bf16 AllGather of k and one of v
    inside each 4-core group; attention runs a uniform 16-slot loop per head
    with per-core causal mask inputs (mask applied multiplicatively to
    exp-scores; a ones-column appended to v makes masked slots drop out of
    both the softmax numerator and denominator).
  - Matmuls in bf16 with fp32 PSUM accumulation; residual stream fp32.
"""

import numpy as np
import ml_dtypes
from contextlib import ExitStack

import concourse.bass as bass
import concourse.tile as tile
from concourse import bacc, mybir
from concourse.bass_utils import run_bass_kernel_spmd

F32 = mybir.dt.float32
BF16 = mybir.dt.bfloat16
AF = mybir.ActivationFunctionType

# model constants (full problem)
B, K, T, D, NH, HD, DFF, L = 2, 512, 2048, 1024, 16, 64, 4096, 2
EPS = 1e-5
G = 4  # cores per batch group

# stream_shuffle mask: swap adjacent partition pairs within each 32-quadrant
PAIR_MASK = [i ^ 1 for i in range(32)]


def build_decoder(T_, S_, D_, NH_, HD_, DFF_, L_, n_cores=8, sim_safe=False):
    """Build the SPMD decoder graph. S_ = tokens per core, T_ = total tokens
    per batch. Group size G divides cores into batch groups."""
    Dt = D_ // 128          # D partition-tiles
    QT = (NH_ * HD_) // 128  # head-pair tiles (2 heads per tile)
    KT = T_ // 128          # k-slots per head
    TT = S_ // 128          # local token tiles
    CH = max(1, DFF_ // 1024)   # mlp chunks
    DFT = (DFF_ // CH) // 128   # dff tiles per chunk (8)
    WV = min(512, NH_ * HD_)    # v-proj psum width
    VH = (NH_ * HD_) // WV      # v-proj col halves
    HPV = WV // HD_             # heads per v-psum
    scale = 1.0 / float(np.sqrt(HD_))
    n_groups = n_cores // G
    rg = [list(range(g * G, (g + 1) * G)) for g in range(n_groups)]

    nc = bacc.Bacc("TRN2", target_bir_lowering=False, debug=False,
                   num_devices=n_cores)

    # ---- I/O ----
    x0T = nc.dram_tensor("x0T", [D_, S_], F32, kind="ExternalInput")
    wqk = nc.dram_tensor("wqk", [L_, 2 * QT, Dt, 128, 128], BF16,
                         kind="ExternalInput")  # packed q|k lhsT tiles
    wv = nc.dram_tensor("wv", [L_, D_, D_], BF16, kind="ExternalInput")
    wo = nc.dram_tensor("wo", [L_, Dt, QT, 128, 128], BF16,
                        kind="ExternalInput")  # packed per-dt slabs
    w13 = nc.dram_tensor("w13", [L_, CH, DFT, 2, Dt, 128, 128], BF16,
                         kind="ExternalInput")
    w2 = nc.dram_tensor("w2", [L_, CH, Dt, DFT, 128, 128], BF16,
                        kind="ExternalInput")
    cosP = nc.dram_tensor("cosP", [128, S_], BF16, kind="ExternalInput")
    sinP = nc.dram_tensor("sinP", [128, S_], BF16, kind="ExternalInput")
    masks = nc.dram_tensor("masks", [KT, 128, S_], BF16, kind="ExternalInput")
    fw = nc.dram_tensor("fw", [128, Dt], F32, kind="ExternalInput")
    out = nc.dram_tensor("out", [D_, S_], F32, kind="ExternalOutput")

    with tile.TileContext(nc) as tc, ExitStack() as ctx:
        # ---- pools ----
        singles = ctx.enter_context(tc.tile_pool(name="singles", bufs=1))
        wqk_p = ctx.enter_context(tc.tile_pool(name="wqk", bufs=3))
        wv_p = ctx.enter_context(tc.tile_pool(name="wv", bufs=Dt))
        wo_p = ctx.enter_context(tc.tile_pool(name="wo", bufs=3))
        w13_p = ctx.enter_context(tc.tile_pool(name="w13", bufs=3))
        w2_p = ctx.enter_context(tc.tile_pool(name="w2", bufs=3))
        h_p = ctx.enter_context(tc.tile_pool(name="h", bufs=Dt))
        q_p = ctx.enter_context(tc.tile_pool(name="q", bufs=QT))
        vstg_p = ctx.enter_context(tc.tile_pool(name="vstg", bufs=TT))
        kslab_p = ctx.enter_context(tc.tile_pool(name="kslab", bufs=2))
        vslab_p = ctx.enter_context(tc.tile_pool(name="vslab", bufs=3))
        e_p = ctx.enter_context(tc.tile_pool(name="e", bufs=8))
        tmp_p = ctx.enter_context(tc.tile_pool(name="tmp", bufs=2))
        oh_p = ctx.enter_context(tc.tile_pool(name="oh", bufs=QT))
        prod_p = ctx.enter_context(tc.tile_pool(name="prod", bufs=DFT + 1))
        oacc_p = ctx.enter_context(tc.tile_pool(name="oacc", bufs=Dt))
        sq_p = ctx.enter_context(tc.tile_pool(name="sq", bufs=3))
        small_p = ctx.enter_context(tc.tile_pool(name="small", bufs=2))
        ps_mm = ctx.enter_context(tc.tile_pool(name="ps_mm", bufs=4,
                                               space="PSUM"))
        ps_av = ctx.enter_context(tc.tile_pool(name="ps_av", bufs=2,
                                               space="PSUM"))
        ps_ms = ctx.enter_context(tc.tile_pool(name="ps_ms", bufs=2,
                                               space="PSUM"))
        dram = ctx.enter_context(tc.tile_pool(name="dram", bufs=2,
                                              space="DRAM"))

        # ---- persistent constants ----
        ones_col = singles.tile([128, 1], BF16, tag="ones_col")
        nc.vector.memset(ones_col[:], 1.0)
        eps_sb = singles.tile([1, 1], F32, tag="eps")
        nc.vector.memset(eps_sb[:], EPS)
        ones_row = singles.tile([1, 128], BF16, tag="ones_row")
        nc.vector.memset(ones_row[:], 1.0)
        cos_sb = singles.tile([128, S_], BF16, tag="cos")
        nc.sync.dma_start(cos_sb[:], cosP[:])
        sin_sb = singles.tile([128, S_], BF16, tag="sin")
        nc.sync.dma_start(sin_sb[:], sinP[:])
        fw_sb = singles.tile([128, Dt], F32, tag="fw")
        nc.sync.dma_start(fw_sb[:], fw[:])
        mask_sb = []
        for s in range(KT):
            m = singles.tile([128, S_], BF16, tag=f"mask{s}")
            nc.sync.dma_start(m[:], masks[s])
            mask_sb.append(m)
        x_sb = []
        for t in range(Dt):
            xt = singles.tile([128, S_], F32, tag=f"x{t}")
            nc.sync.dma_start(xt[:], x0T[t * 128:(t + 1) * 128, :])
            x_sb.append(xt)

        def rmsnorm_bcast(x_tiles):
            """Return a PSUM tile [128, S] holding rstd broadcast to all
            partitions (1/sqrt(mean(x^2)+eps) per token column)."""
            ssum = ps_ms.tile([128, S_], F32, tag="misc")
            for t in range(Dt):
                sq = sq_p.tile([128, S_], BF16, tag="sq")
                nc.vector.tensor_mul(sq[:], x_tiles[t][:], x_tiles[t][:])
                nc.tensor.matmul(ssum[0:1, :], ones_col[:], sq[:],
                                 start=(t == 0), stop=(t == Dt - 1))
            rstd = small_p.tile([1, S_], F32, tag="rstd")
            nc.scalar.activation(rstd[:], ssum[0:1, :], AF.Sqrt,
                                 bias=eps_sb[:], scale=1.0 / D_)
            nc.vector.reciprocal(rstd[:], rstd[:])
            rstd_bf = small_p.tile([1, S_], BF16, tag="rstd_bf")
            nc.vector.tensor_copy(rstd_bf[:], rstd[:])
            rn = ps_ms.tile([128, S_], F32, tag="misc")
            nc.tensor.matmul(rn[:], ones_row[:], rstd_bf[:],
                             start=True, stop=True)
            return rn

        def rmsnorm_to_h(x_tiles):
            rn = rmsnorm_bcast(x_tiles)
            hs = []
            for t in range(Dt):
                ht = h_p.tile([128, S_], BF16, tag="h")
                nc.vector.tensor_mul(ht[:], x_tiles[t][:], rn[:])
                hs.append(ht)
            return hs

        def rope_from_psum(ps, dst):
            """dst(bf16) = ps*cos + shuffle(ps)*sin  (pair-permuted RoPE)."""
            qb = tmp_p.tile([128, S_], BF16, tag="qb")
            nc.vector.tensor_copy(qb[:], ps[:])
            shuf = tmp_p.tile([128, S_], BF16, tag="shuf")
            nc.vector.stream_shuffle(shuf[:], qb[:], PAIR_MASK)
            qc = tmp_p.tile([128, S_], BF16, tag="qc")
            nc.vector.tensor_mul(qc[:], qb[:], cos_sb[:])
            nc.vector.tensor_mul(shuf[:], shuf[:], sin_sb[:])
            nc.vector.tensor_add(dst[:], qc[:], shuf[:])

        for l in range(L_):
            # ---------------- attention sublayer ----------------
            h = rmsnorm_to_h(x_sb)

            # k tiles first -> rope -> stage -> kick AllGather early
            kin_k = dram.tile([NH_, HD_, S_], BF16, tag="kin_k")
            for j in range(QT):
                wj = wqk_p.tile([128, Dt * 128], BF16, tag="wqk")
                nc.sync.dma_start(wj.rearrange("p (k c) -> p k c", c=128),
                                  wqk[l, QT + j].rearrange("k p c -> p k c"))
                ps = ps_mm.tile([128, S_], F32, tag="mm")
                for kt in range(Dt):
                    nc.tensor.matmul(ps[:], wj[:, kt * 128:(kt + 1) * 128],
                                     h[kt][:], start=(kt == 0),
                                     stop=(kt == Dt - 1))
                kj = tmp_p.tile([128, S_], BF16, tag="kj")
                rope_from_psum(ps, kj)
                nc.sync.dma_start(
                    kin_k[2 * j:2 * j + 2].rearrange("h d s -> (h d) s"),
                    kj[:])
            kout_k = dram.tile([G, NH_, HD_, S_], BF16, tag="kout_k")
            nc.gpsimd.collective_compute(
                "AllGather", mybir.AluOpType.bypass, replica_groups=rg,
                ins=[kin_k.opt()], outs=[kout_k.opt()])

            # v tiles (token-major) -> stage [head, tok, 65] -> AllGather
            wv_sb = []
            for kt in range(Dt):
                wt = wv_p.tile([128, D_], BF16, tag="wv")
                nc.sync.dma_start(wt[:], wv[l, kt * 128:(kt + 1) * 128, :])
                wv_sb.append(wt)
            kin_v = dram.tile([NH_, S_, 65], BF16, tag="kin_v")
            for tt in range(TT):
                vs = vstg_p.tile([128, NH_ * 65], BF16, tag="vstg")
                for half in range(VH):
                    ps = ps_mm.tile([128, WV], F32, tag="mm")
                    for kt in range(Dt):
                        nc.tensor.matmul(
                            ps[:], h[kt][:, tt * 128:(tt + 1) * 128],
                            wv_sb[kt][:, half * WV:(half + 1) * WV],
                            start=(kt == 0), stop=(kt == Dt - 1))
                    for hh in range(HPV):
                        hd0 = half * HPV + hh
                        nc.vector.tensor_copy(
                            vs[:, hd0 * 65:hd0 * 65 + 64],
                            ps[:, hh * HD_:(hh + 1) * HD_])
                for hd0 in range(NH_):
                    nc.vector.memset(vs[:, hd0 * 65 + 64:hd0 * 65 + 65], 1.0)
                nc.sync.dma_start(
                    kin_v.rearrange("h s c -> s h c")[tt * 128:(tt + 1) * 128],
                    vs.rearrange("p (h c) -> p h c", c=65))
            kout_v = dram.tile([G, NH_, S_, 65], BF16, tag="kout_v")
            nc.gpsimd.collective_compute(
                "AllGather", mybir.AluOpType.bypass, replica_groups=rg,
                ins=[kin_v.opt()], outs=[kout_v.opt()])

            # q tiles (feature-major, rope'd) — overlaps the AllGathers
            q_sb = []
            for j in range(QT):
                wj = wqk_p.tile([128, Dt * 128], BF16, tag="wqk")
                nc.sync.dma_start(wj.rearrange("p (k c) -> p k c", c=128),
                                  wqk[l, j].rearrange("k p c -> p k c"))
                ps = ps_mm.tile([128, S_], F32, tag="mm")
                for kt in range(Dt):
                    nc.tensor.matmul(ps[:], wj[:, kt * 128:(kt + 1) * 128],
                                     h[kt][:], start=(kt == 0),
                                     stop=(kt == Dt - 1))
                qj = q_p.tile([128, S_], BF16, tag="q")
                rope_from_psum(ps, qj)
                q_sb.append(qj)

            # attention, one head PAIR at a time: the two K=64 score matmuls
            # target distinct PE row groups (tile_position auto-derived from
            # the lhsT partition base) and run concurrently in the array
            oh_sb = []
            for j in range(QT):
                kpair = kslab_p.tile([128, KT * 128], BF16, tag="kslab")
                for hh in range(2):
                    nc.sync.dma_start(
                        kpair[hh * 64:(hh + 1) * 64, :].rearrange(
                            "d (g s) -> d g s", g=G),
                        kout_k[:, 2 * j + hh].rearrange("g d s -> d g s"))
                vsl = []
                for hh in range(2):
                    vslab = vslab_p.tile([128, KT, 65], BF16, tag="vslab")
                    for g in range(G):
                        nc.sync.dma_start(
                            vslab[:, g * TT:(g + 1) * TT, :],
                            kout_v[g, 2 * j + hh].rearrange(
                                "(t p) c -> p t c", p=128))
                    vsl.append(vslab)
                av = [ps_av.tile([65, S_], F32, tag="av", name=f"av{j}_{hh2}")
                      for hh2 in range(2)]
                # software pipeline: emit score-matmuls LEAD slots ahead of
                # the AV accumulation so the in-order PE never stalls on the
                # exp->mask round trip
                LEAD = 2
                es_q = []
                for s in range(KT):
                    es = []
                    for hh in range(2):
                        sc = ps_mm.tile([128, S_], F32, tag="mm")
                        nc.tensor.matmul(sc[:],
                                         kpair[hh * 64:hh * 64 + 64,
                                               s * 128:(s + 1) * 128],
                                         q_sb[j][hh * 64:hh * 64 + 64, :],
                                         start=True, stop=True)
                        e = e_p.tile([128, S_], BF16, tag="e")
                        nc.scalar.activation(e[:], sc[:], AF.Exp, scale=scale)
                        nc.vector.tensor_mul(e[:], e[:], mask_sb[s][:])
                        es.append(e)
                    es_q.append(es)
                    if s >= LEAD:
                        s2 = s - LEAD
                        for hh in range(2):
                            nc.tensor.matmul(av[hh][:], vsl[hh][:, s2, :],
                                             es_q[s2][hh][:], start=(s2 == 0),
                                             stop=(s2 == KT - 1))
                for s2 in range(KT - LEAD, KT):
                    for hh in range(2):
                        nc.tensor.matmul(av[hh][:], vsl[hh][:, s2, :],
                                         es_q[s2][hh][:], start=(s2 == 0),
                                         stop=(s2 == KT - 1))
                ohp = oh_p.tile([128, S_], BF16, tag="oh")
                oh_sb.append(ohp)
                for hh in range(2):
                    recip = small_p.tile([1, S_], F32, tag="recip")
                    nc.vector.reciprocal(recip[:], av[hh][64:65, :])
                    recip_bf = small_p.tile([1, S_], BF16, tag="recip_bf")
                    nc.vector.tensor_copy(recip_bf[:], recip[:])
                    rb = ps_ms.tile([128, S_], F32, tag="misc")
                    nc.tensor.matmul(rb[0:64, :], ones_row[:, 0:64],
                                     recip_bf[:], start=True, stop=True)
                    rb_sb = tmp_p.tile([64, S_], BF16, tag="rb_sb")
                    nc.vector.tensor_copy(rb_sb[:], rb[0:64, :])
                    # odd head writes the upper partition half (64-ch DVE ops
                    # may target either half)
                    nc.vector.tensor_mul(ohp[hh * 64:hh * 64 + 64, :],
                                         av[hh][0:64, :], rb_sb[:])

            # o-projection + residual (K=128 per head pair)
            for dt in range(Dt):
                wos = wo_p.tile([128, QT * 128], BF16, tag="wo")
                nc.sync.dma_start(wos.rearrange("p (j c) -> p j c", c=128),
                                  wo[l, dt].rearrange("j p c -> p j c"))
                ps = ps_mm.tile([128, S_], F32, tag="mm")
                for jp in range(QT):
                    nc.tensor.matmul(ps[:], wos[:, jp * 128:(jp + 1) * 128],
                                     oh_sb[jp][:], start=(jp == 0),
                                     stop=(jp == QT - 1))
                nc.vector.tensor_add(x_sb[dt][:], x_sb[dt][:], ps[:])

            # ---------------- mlp sublayer ----------------
            h2 = rmsnorm_to_h(x_sb)
            oacc = []
            for ch in range(CH):
                prods = []
                for df in range(DFT):
                    w13s = w13_p.tile([128, 2 * Dt * 128], BF16, tag="w13")
                    nc.sync.dma_start(
                        w13s.rearrange("p (u k c) -> p u k c", u=2, c=128),
                        w13[l, ch, df].rearrange("u k p c -> p u k c"))
                    gps = ps_mm.tile([128, S_], F32, tag="mm")
                    ups = ps_mm.tile([128, S_], F32, tag="mm")
                    for kt in range(Dt):
                        nc.tensor.matmul(gps[:],
                                         w13s[:, kt * 128:(kt + 1) * 128],
                                         h2[kt][:], start=(kt == 0),
                                         stop=(kt == Dt - 1))
                    for kt in range(Dt):
                        off = Dt * 128
                        nc.tensor.matmul(
                            ups[:], w13s[:, off + kt * 128:off + (kt + 1) * 128],
                            h2[kt][:], start=(kt == 0), stop=(kt == Dt - 1))
                    gs = e_p.tile([128, S_], BF16, tag="e")
                    pr = prod_p.tile([128, S_], BF16, tag="prod")
                    if sim_safe:
                        # CoreSim lacks Silu; sigmoid + explicit mul
                        nc.scalar.activation(gs[:], gps[:], AF.Sigmoid)
                        gg = tmp_p.tile([128, S_], BF16, tag="gg")
                        nc.vector.tensor_mul(gg[:], gs[:], gps[:])
                        nc.vector.tensor_mul(pr[:], gg[:], ups[:])
                    else:
                        nc.scalar.activation(gs[:], gps[:], AF.Silu)
                        nc.vector.tensor_mul(pr[:], gs[:], ups[:])
                    prods.append(pr)
                for dt in range(Dt):
                    w2s = w2_p.tile([128, DFT * 128], BF16, tag="w2")
                    nc.sync.dma_start(
                        w2s.rearrange("p (j c) -> p j c", c=128),
                        w2[l, ch, dt].rearrange("j p c -> p j c"))
                    ps = ps_mm.tile([128, S_], F32, tag="mm")
                    for j in range(DFT):
                        nc.tensor.matmul(ps[:], w2s[:, j * 128:(j + 1) * 128],
                                         prods[j][:], start=(j == 0),
                                         stop=(j == DFT - 1))
                    if ch == 0:
                        oa = oacc_p.tile([128, S_], F32, tag="oacc")
                        nc.vector.tensor_copy(oa[:], ps[:])
                        oacc.append(oa)
                    else:
                        nc.vector.tensor_add(oacc[dt][:], oacc[dt][:], ps[:])
            for dt in range(Dt):
                nc.vector.tensor_add(x_sb[dt][:], x_sb[dt][:], oacc[dt][:])

        # ---------------- final rmsnorm ----------------
        rn = rmsnorm_bcast(x_sb)
        for dt in range(Dt):
            xn = tmp_p.tile([128, S_], F32, tag="xn")
            nc.vector.tensor_mul(xn[:], x_sb[dt][:], rn[:])
            nc.vector.tensor_scalar_mul(xn[:], xn[:], fw_sb[:, dt:dt + 1])
            nc.sync.dma_start(out[dt * 128:(dt + 1) * 128, :], xn[:])

    nc.compile()
    return nc


# ---------------------------------------------------------------------------
# host-side preparation
# ---------------------------------------------------------------------------

def _bf16(a):
    return np.ascontiguousarray(np.asarray(a, dtype=np.float32)).astype(
        ml_dtypes.bfloat16)


def _perm(HD_):
    """Head-dim pair permutation: slot 2i <- dim i, slot 2i+1 <- dim i+HD/2."""
    half = HD_ // 2
    p = np.empty(HD_, dtype=np.int64)
    p[0::2] = np.arange(half)
    p[1::2] = np.arange(half) + half
    return p


def prepare_in_maps(x0, cos, sin, wq, wk, wv_, wo_, anw, mnw, w1, w3, w2_,
                    fnw, T_, S_, D_, NH_, HD_, DFF_, L_, n_cores=8):
    """Build per-core input dicts. x0 is the already-expanded [B', T, D] fp32
    input (B' = n_cores // G batches)."""
    Dt = D_ // 128
    QT = (NH_ * HD_) // 128
    KT = T_ // 128
    CH = max(1, DFF_ // 1024)
    DFT = (DFF_ // CH) // 128
    perm = _perm(HD_)
    half = HD_ // 2

    # fold norm weights into the consuming projections
    wq_e = anw[:, :, None] * wq      # [L, D, D]
    wk_e = anw[:, :, None] * wk
    wv_e = anw[:, :, None] * wv_
    w1_e = mnw[:, :, None] * w1      # [L, D, DFF]
    w3_e = mnw[:, :, None] * w3

    # permute q/k columns per head by `perm`
    def permute_cols(w):
        wh = w.reshape(L_, D_, NH_, HD_)
        return wh[:, :, :, perm].reshape(L_, D_, NH_ * HD_)

    wq_p = permute_cols(wq_e)
    wk_p = permute_cols(wk_e)

    # packed q|k lhsT tiles: [L, 2QT, Dt, 128, 128]
    wqk_pack = np.empty((L_, 2 * QT, Dt, 128, 128), dtype=np.float32)
    for j in range(QT):
        for kt in range(Dt):
            wqk_pack[:, j, kt] = wq_p[:, kt * 128:(kt + 1) * 128,
                                      j * 128:(j + 1) * 128]
            wqk_pack[:, QT + j, kt] = wk_p[:, kt * 128:(kt + 1) * 128,
                                           j * 128:(j + 1) * 128]
    # wo packed: [L, Dt, QT, 128, 128]; rows = o dims (head-major)
    wo_pack = np.empty((L_, Dt, QT, 128, 128), dtype=np.float32)
    for dt in range(Dt):
        for j in range(QT):
            wo_pack[:, dt, j] = wo_[:, j * 128:(j + 1) * 128,
                                    dt * 128:(dt + 1) * 128]
    # w13 packed: [L, CH, DFT, 2, Dt, 128, 128]
    csz = DFF_ // CH
    w13_pack = np.empty((L_, CH, DFT, 2, Dt, 128, 128), dtype=np.float32)
    for ch in range(CH):
        for df in range(DFT):
            c0 = ch * csz + df * 128
            for kt in range(Dt):
                w13_pack[:, ch, df, 0, kt] = w1_e[:, kt * 128:(kt + 1) * 128,
                                                  c0:c0 + 128]
                w13_pack[:, ch, df, 1, kt] = w3_e[:, kt * 128:(kt + 1) * 128,
                                                  c0:c0 + 128]
    # w2 packed: [L, CH, Dt, DFT, 128, 128]
    w2_pack = np.empty((L_, CH, Dt, DFT, 128, 128), dtype=np.float32)
    for ch in range(CH):
        for dt in range(Dt):
            for j in range(DFT):
                r0 = ch * csz + j * 128
                w2_pack[:, ch, dt, j] = w2_[:, r0:r0 + 128,
                                            dt * 128:(dt + 1) * 128]

    wqk_b = _bf16(wqk_pack)
    wv_b = _bf16(wv_e)
    wo_b = _bf16(wo_pack)
    w13_b = _bf16(w13_pack)
    w2_b = _bf16(w2_pack)
    fw_np = np.ascontiguousarray(
        np.asarray(fnw, np.float32).reshape(Dt, 128).T)  # [128, Dt]

    # rope tables, permuted + sign-baked, duplicated per head pair -> [128, T]
    cosPf = np.asarray(cos, np.float32)[:, perm].T        # [HD, T]
    sinf = np.asarray(sin, np.float32)[:, perm].T         # [HD, T]
    sign = np.where(np.arange(HD_) % 2 == 0, -1.0, 1.0)[:, None]
    sinPf = sinf * sign
    cosP2 = np.tile(cosPf, (2, 1))                        # [128, T]
    sinP2 = np.tile(sinPf, (2, 1))

    in_maps = []
    for c in range(n_cores):
        b = c // G
        r = c % G
        t0 = r * S_
        xs = np.ascontiguousarray(x0[b, t0:t0 + S_, :].T).astype(np.float32)
        mask = np.zeros((KT, 128, S_), dtype=np.float32)
        for s in range(KT):
            kg = 128 * s + np.arange(128)[:, None]
            qg = t0 + np.arange(S_)[None, :]
            mask[s] = (kg <= qg).astype(np.float32)
        in_maps.append({
            "x0T": xs,
            "wqk": wqk_b, "wv": wv_b, "wo": wo_b, "w13": w13_b, "w2": w2_b,
            "cosP": _bf16(cosP2[:, t0:t0 + S_]),
            "sinP": _bf16(sinP2[:, t0:t0 + S_]),
            "masks": mask.astype(ml_dtypes.bfloat16),
            "fw": fw_np,
        })
    return in_maps


def expand_input(x_processed, boundaries, counts, x_residual):
    """Ragged chunk expansion: token t of batch b takes chunk
    #{boundaries[b] <= t}, plus residual."""
    xp = np.asarray(x_processed, np.float32)
    bd = np.asarray(boundaries)
    xr = np.asarray(x_residual, np.float32)
    Bn, Tn, Dn = xr.shape
    tt = np.arange(Tn)
    out = np.empty_like(xr)
    for b in range(Bn):
        idx = np.searchsorted(bd[b], tt, side="right")
        out[b] = xp[b, idx, :] + xr[b]
    return out


_NC_CACHE = {}


def _get_nc(key):
    if key not in _NC_CACHE:
        _NC_CACHE[key] = build_decoder(*key)
    return _NC_CACHE[key]


def kernel(x_processed, boundaries, counts, x_residual, cos, sin, seq_len,
           wq, wk, wv, wo, attn_norm_w, mlp_norm_w, w1, w3, w2, final_norm_w,
           _trace=False):
    S_ = T // G
    x0 = expand_input(x_processed, boundaries, counts, x_residual)
    in_maps = prepare_in_maps(
        x0, cos, sin,
        np.asarray(wq, np.float32), np.asarray(wk, np.float32),
        np.asarray(wv, np.float32), np.asarray(wo, np.float32),
        np.asarray(attn_norm_w, np.float32), np.asarray(mlp_norm_w, np.float32),
        np.asarray(w1, np.float32), np.asarray(w3, np.float32),
        np.asarray(w2, np.float32), np.asarray(final_norm_w, np.float32),
        T, S_, D, NH, HD, DFF, L, n_cores=8)
    nc = _get_nc((T, S_, D, NH, HD, DFF, L, 8))
    res = run_bass_kernel_spmd(nc, in_maps, list(range(8)), trace=_trace)
    outp = np.empty((B, T, D), dtype=np.float32)
    for c in range(8):
        b, r = c // G, c % G
        outp[b, r * S_:(r + 1) * S_, :] = res.results[c]["out"].T
    if _trace:
        kernel.last_exec_time_ns = res.exec_time_ns
        kernel.last_results = res
    return outp


# revision 18
# speedup vs baseline: 1.0997x; 1.0896x over previous
"""Trainium2 Bass kernel for nn_Decoder_10230612099842.

2-layer decoder (rmsnorm / qkv+RoPE / causal attention / o-proj / rmsnorm /
silu-gated MLP / final rmsnorm) over a ragged-chunk-expanded input.

Strategy (8 NeuronCores = one TRN2 chip):
  - Host: ragged chunk expansion (searchsorted gather) + residual add, weight
    norm-folding, bf16 cast, head-dim pair-permutation for RoPE, per-core
    slicing.
  - Device: data-parallel over batch (2 groups of 4 cores), sequence-parallel
    over tokens within a group (512 tokens/core). Activations live
    feature-major [D, S].  Per layer, one bf16 AllGather of k and one of v
    inside each 4-core group; attention runs a uniform slot loop per head
    with per-core causal mask inputs (mask applied multiplicatively to
    exp-scores; a ones-column appended to v makes masked slots drop out of
    both the softmax numerator and denominator).  Scores for two k-slots
    share one 2-bank PSUM tile so exp/mask run as single wide ops, and the
    score matmuls are emitted one slot-pair ahead of the AV accumulation so
    the in-order PE never waits on the exp round-trip.
  - Matmuls in bf16 with fp32 PSUM accumulation; residual stream fp32.
"""

import numpy as np
import ml_dtypes
from contextlib import ExitStack

import concourse.bass as bass
import concourse.tile as tile
from concourse import bacc, mybir
from concourse.bass_utils import run_bass_kernel_spmd

F32 = mybir.dt.float32
BF16 = mybir.dt.bfloat16
AF = mybir.ActivationFunctionType

# model constants (full problem)
B, K, T, D, NH, HD, DFF, L = 2, 512, 2048, 1024, 16, 64, 4096, 2
EPS = 1e-5
G = 4  # cores per batch group

# stream_shuffle mask: swap adjacent partition pairs within each 32-quadrant
PAIR_MASK = [i ^ 1 for i in range(32)]


def build_decoder(T_, S_, D_, NH_, HD_, DFF_, L_, n_cores=8, sim_safe=False):
    """Build the SPMD decoder graph. S_ = tokens per core, T_ = total tokens
    per batch. Group size G divides cores into batch groups."""
    Dt = D_ // 128          # D partition-tiles
    QT = (NH_ * HD_) // 128  # head-pair tiles (2 heads per tile)
    KT = T_ // 128          # k-slots per head
    KP = KT // 2            # k-slot pairs
    TT = S_ // 128          # local token tiles
    CH = max(1, DFF_ // 1024)   # mlp chunks
    DFT = (DFF_ // CH) // 128   # dff tiles per chunk (8)
    WV = min(512, NH_ * HD_)    # v-proj psum width
    VH = (NH_ * HD_) // WV      # v-proj col halves
    HPV = WV // HD_             # heads per v-psum
    scale = 1.0 / float(np.sqrt(HD_))
    n_groups = n_cores // G
    rg = [list(range(g * G, (g + 1) * G)) for g in range(n_groups)]

    nc = bacc.Bacc("TRN2", target_bir_lowering=False, debug=False,
                   num_devices=n_cores)

    # ---- I/O ----
    x0T = nc.dram_tensor("x0T", [D_, S_], F32, kind="ExternalInput")
    wqk = nc.dram_tensor("wqk", [L_, 2 * QT, Dt, 128, 128], BF16,
                         kind="ExternalInput")  # packed q|k lhsT tiles
    wv = nc.dram_tensor("wv", [L_, D_, D_], BF16, kind="ExternalInput")
    wo = nc.dram_tensor("wo", [L_, Dt, QT, 128, 128], BF16,
                        kind="ExternalInput")  # packed per-dt slabs
    w13 = nc.dram_tensor("w13", [L_, CH, DFT, 2, Dt, 128, 128], BF16,
                         kind="ExternalInput")
    w2 = nc.dram_tensor("w2", [L_, CH, Dt, DFT, 128, 128], BF16,
                        kind="ExternalInput")
    cosP = nc.dram_tensor("cosP", [128, S_], BF16, kind="ExternalInput")
    sinP = nc.dram_tensor("sinP", [128, S_], BF16, kind="ExternalInput")
    masks = nc.dram_tensor("masks", [KP, 128, 2 * S_], BF16,
                           kind="ExternalInput")  # slot-pair packed
    fw = nc.dram_tensor("fw", [128, Dt], F32, kind="ExternalInput")
    out = nc.dram_tensor("out", [D_, S_], F32, kind="ExternalOutput")

    with tile.TileContext(nc) as tc, ExitStack() as ctx:
        # ---- pools ----
        singles = ctx.enter_context(tc.tile_pool(name="singles", bufs=1))
        wqk_p = ctx.enter_context(tc.tile_pool(name="wqk", bufs=3))
        wv_p = ctx.enter_context(tc.tile_pool(name="wv", bufs=Dt))
        wo_p = ctx.enter_context(tc.tile_pool(name="wo", bufs=3))
        w13_p = ctx.enter_context(tc.tile_pool(name="w13", bufs=3))
        w2_p = ctx.enter_context(tc.tile_pool(name="w2", bufs=3))
        h_p = ctx.enter_context(tc.tile_pool(name="h", bufs=Dt))
        q_p = ctx.enter_context(tc.tile_pool(name="q", bufs=QT))
        vstg_p = ctx.enter_context(tc.tile_pool(name="vstg", bufs=TT))
        kslab_p = ctx.enter_context(tc.tile_pool(name="kslab", bufs=2))
        vslab_p = ctx.enter_context(tc.tile_pool(name="vslab", bufs=3))
        e_p = ctx.enter_context(tc.tile_pool(name="e", bufs=4))
        tmp_p = ctx.enter_context(tc.tile_pool(name="tmp", bufs=2))
        oh_p = ctx.enter_context(tc.tile_pool(name="oh", bufs=QT))
        avs_p = ctx.enter_context(tc.tile_pool(name="avs", bufs=4))
        prod_p = ctx.enter_context(tc.tile_pool(name="prod", bufs=DFT + 1))
        oacc_p = ctx.enter_context(tc.tile_pool(name="oacc", bufs=Dt))
        sq_p = ctx.enter_context(tc.tile_pool(name="sq", bufs=3))
        small_p = ctx.enter_context(tc.tile_pool(name="small", bufs=2))
        ps_big = ctx.enter_context(tc.tile_pool(name="ps_big", bufs=3,
                                                space="PSUM"))
        ps_av = ctx.enter_context(tc.tile_pool(name="ps_av", bufs=2,
                                               space="PSUM"))
        dram = ctx.enter_context(tc.tile_pool(name="dram", bufs=2,
                                              space="DRAM"))

        # ---- persistent constants ----
        ones_col = singles.tile([128, 1], BF16, tag="ones_col")
        nc.vector.memset(ones_col[:], 1.0)
        eps_sb = singles.tile([1, 1], F32, tag="eps")
        nc.vector.memset(eps_sb[:], EPS)
        ones_row = singles.tile([1, 128], BF16, tag="ones_row")
        nc.vector.memset(ones_row[:], 1.0)
        cos_sb = singles.tile([128, S_], BF16, tag="cos")
        nc.sync.dma_start(cos_sb[:], cosP[:])
        sin_sb = singles.tile([128, S_], BF16, tag="sin")
        nc.sync.dma_start(sin_sb[:], sinP[:])
        fw_sb = singles.tile([128, Dt], F32, tag="fw")
        nc.sync.dma_start(fw_sb[:], fw[:])
        mask_sb = []
        for p in range(KP):
            m = singles.tile([128, 2 * S_], BF16, tag=f"mask{p}")
            nc.sync.dma_start(m[:], masks[p])
            mask_sb.append(m)
        x_sb = []
        for t in range(Dt):
            xt = singles.tile([128, S_], F32, tag=f"x{t}")
            nc.sync.dma_start(xt[:], x0T[t * 128:(t + 1) * 128, :])
            x_sb.append(xt)

        def rmsnorm_bcast(x_tiles):
            """Return a PSUM tile [128, S] holding rstd broadcast to all
            partitions (1/sqrt(mean(x^2)+eps) per token column)."""
            ssum = ps_av.tile([128, S_], F32, tag="av")
            for t in range(Dt):
                sq = sq_p.tile([128, S_], BF16, tag="sq")
                nc.vector.tensor_mul(sq[:], x_tiles[t][:], x_tiles[t][:])
                nc.tensor.matmul(ssum[0:1, :], ones_col[:], sq[:],
                                 start=(t == 0), stop=(t == Dt - 1))
            rstd = small_p.tile([1, S_], F32, tag="rstd")
            nc.scalar.activation(rstd[:], ssum[0:1, :], AF.Sqrt,
                                 bias=eps_sb[:], scale=1.0 / D_)
            nc.vector.reciprocal(rstd[:], rstd[:])
            rstd_bf = small_p.tile([1, S_], BF16, tag="rstd_bf")
            nc.vector.tensor_copy(rstd_bf[:], rstd[:])
            rn = ps_av.tile([128, S_], F32, tag="av")
            nc.tensor.matmul(rn[:], ones_row[:], rstd_bf[:],
                             start=True, stop=True)
            return rn

        def rmsnorm_to_h(x_tiles):
            rn = rmsnorm_bcast(x_tiles)
            hs = []
            for t in range(Dt):
                ht = h_p.tile([128, S_], BF16, tag="h")
                nc.vector.tensor_mul(ht[:], x_tiles[t][:], rn[:])
                hs.append(ht)
            return hs

        def rope_from_psum(ps, dst):
            """dst(bf16) = ps*cos + shuffle(ps)*sin  (pair-permuted RoPE)."""
            qb = tmp_p.tile([128, S_], BF16, tag="qb")
            nc.vector.tensor_copy(qb[:], ps[:])
            shuf = tmp_p.tile([128, S_], BF16, tag="shuf")
            nc.vector.stream_shuffle(shuf[:], qb[:], PAIR_MASK)
            qc = tmp_p.tile([128, S_], BF16, tag="qc")
            nc.vector.tensor_mul(qc[:], qb[:], cos_sb[:])
            nc.vector.tensor_mul(shuf[:], shuf[:], sin_sb[:])
            nc.vector.tensor_add(dst[:], qc[:], shuf[:])

        for l in range(L_):
            # ---------------- attention sublayer ----------------
            h = rmsnorm_to_h(x_sb)

            # k tiles first -> rope -> stage -> kick AllGather early
            kin_k = dram.tile([NH_, HD_, S_], BF16, tag="kin_k")
            for j in range(QT):
                wj = wqk_p.tile([128, Dt * 128], BF16, tag="wqk")
                nc.gpsimd.dma_start(wj.rearrange("p (k c) -> p k c", c=128),
                                    wqk[l, QT + j].rearrange("k p c -> p k c"))
                ps = ps_big.tile([128, 2 * S_], F32, tag="big")
                for kt in range(Dt):
                    nc.tensor.matmul(ps[:, 0:S_],
                                     wj[:, kt * 128:(kt + 1) * 128],
                                     h[kt][:], start=(kt == 0),
                                     stop=(kt == Dt - 1))
                kj = tmp_p.tile([128, S_], BF16, tag="kj")
                rope_from_psum(ps[:, 0:S_], kj)
                nc.sync.dma_start(
                    kin_k[2 * j:2 * j + 2].rearrange("h d s -> (h d) s"),
                    kj[:])
            kout_k = dram.tile([G, NH_, HD_, S_], BF16, tag="kout_k")
            nc.gpsimd.collective_compute(
                "AllGather", mybir.AluOpType.bypass, replica_groups=rg,
                ins=[kin_k.opt()], outs=[kout_k.opt()])

            # v tiles (token-major) -> stage [head, tok, 65] -> AllGather
            wv_sb = []
            for kt in range(Dt):
                wt = wv_p.tile([128, D_], BF16, tag="wv")
                nc.gpsimd.dma_start(wt[:], wv[l, kt * 128:(kt + 1) * 128, :])
                wv_sb.append(wt)
            kin_v = dram.tile([NH_, S_, 65], BF16, tag="kin_v")
            for tt in range(TT):
                vs = vstg_p.tile([128, NH_ * 65], BF16, tag="vstg")
                for half in range(VH):
                    ps = ps_big.tile([128, 2 * S_], F32, tag="big")
                    for kt in range(Dt):
                        nc.tensor.matmul(
                            ps[:, 0:WV], h[kt][:, tt * 128:(tt + 1) * 128],
                            wv_sb[kt][:, half * WV:(half + 1) * WV],
                            start=(kt == 0), stop=(kt == Dt - 1))
                    for hh in range(HPV):
                        hd0 = half * HPV + hh
                        nc.vector.tensor_copy(
                            vs[:, hd0 * 65:hd0 * 65 + 64],
                            ps[:, hh * HD_:(hh + 1) * HD_])
                for hd0 in range(NH_):
                    nc.vector.memset(vs[:, hd0 * 65 + 64:hd0 * 65 + 65], 1.0)
                nc.sync.dma_start(
                    kin_v.rearrange("h s c -> s h c")[tt * 128:(tt + 1) * 128],
                    vs.rearrange("p (h c) -> p h c", c=65))
            kout_v = dram.tile([G, NH_, S_, 65], BF16, tag="kout_v")
            nc.gpsimd.collective_compute(
                "AllGather", mybir.AluOpType.bypass, replica_groups=rg,
                ins=[kin_v.opt()], outs=[kout_v.opt()])

            # q tiles (feature-major, rope'd) — overlaps the AllGathers
            q_sb = []
            for j in range(QT):
                wj = wqk_p.tile([128, Dt * 128], BF16, tag="wqk")
                nc.gpsimd.dma_start(wj.rearrange("p (k c) -> p k c", c=128),
                                    wqk[l, j].rearrange("k p c -> p k c"))
                ps = ps_big.tile([128, 2 * S_], F32, tag="big")
                for kt in range(Dt):
                    nc.tensor.matmul(ps[:, 0:S_],
                                     wj[:, kt * 128:(kt + 1) * 128],
                                     h[kt][:], start=(kt == 0),
                                     stop=(kt == Dt - 1))
                qj = q_p.tile([128, S_], BF16, tag="q")
                rope_from_psum(ps[:, 0:S_], qj)
                q_sb.append(qj)

            # attention, one head PAIR at a time; two k-slots share one
            # 2-bank PSUM tile so exp/mask are single wide ops; score matmuls
            # run one slot-pair ahead of the AV accumulation
            oh_sb = []
            for j in range(QT):
                kpair = kslab_p.tile([128, KT * 128], BF16, tag="kslab")
                for hh in range(2):
                    nc.sync.dma_start(
                        kpair[hh * 64:(hh + 1) * 64, :].rearrange(
                            "d (g s) -> d g s", g=G),
                        kout_k[:, 2 * j + hh].rearrange("g d s -> d g s"))
                vsl = []
                for hh in range(2):
                    vslab = vslab_p.tile([128, KT, 65], BF16, tag="vslab",
                                         name=f"vslab{j}_{hh}")
                    for g in range(G):
                        nc.sync.dma_start(
                            vslab[:, g * TT:(g + 1) * TT, :],
                            kout_v[g, 2 * j + hh].rearrange(
                                "(t p) c -> p t c", p=128))
                    vsl.append(vslab)
                av = [ps_av.tile([128, S_], F32, tag="av", name=f"av{j}_{x}")
                      for x in range(2)]
                es_q = []
                for p in range(KP):
                    es = []
                    for hh in range(2):
                        sc2 = ps_big.tile([128, 2 * S_], F32, tag="big")
                        for u in range(2):
                            s = 2 * p + u
                            nc.tensor.matmul(
                                sc2[:, u * S_:(u + 1) * S_],
                                kpair[hh * 64:hh * 64 + 64,
                                      s * 128:(s + 1) * 128],
                                q_sb[j][hh * 64:hh * 64 + 64, :],
                                start=True, stop=True)
                        e2 = e_p.tile([128, 2 * S_], BF16, tag="e")
                        nc.scalar.activation(e2[:], sc2[:], AF.Exp,
                                             scale=scale)
                        nc.vector.tensor_mul(e2[:], e2[:], mask_sb[p][:])
                        es.append(e2)
                    es_q.append(es)
                    if p >= 1:
                        for hh in range(2):
                            for u in range(2):
                                s = 2 * (p - 1) + u
                                nc.tensor.matmul(
                                    av[hh][0:65, :], vsl[hh][:, s, :],
                                    es_q[p - 1][hh][:, u * S_:(u + 1) * S_],
                                    start=(s == 0), stop=(s == KT - 1))
                for hh in range(2):
                    for u in range(2):
                        s = 2 * (KP - 1) + u
                        nc.tensor.matmul(
                            av[hh][0:65, :], vsl[hh][:, s, :],
                            es_q[KP - 1][hh][:, u * S_:(u + 1) * S_],
                            start=(s == 0), stop=(s == KT - 1))
                # fast-evict AV (frees the PSUM banks), normalize off the
                # critical path: one reciprocal per pair + PE broadcast
                den2 = small_p.tile([1, 2 * S_], F32, tag="den2")
                avs = []
                for hh in range(2):
                    a = avs_p.tile([64, S_], BF16, tag="avs",
                                   name=f"avs{j}_{hh}")
                    nc.vector.tensor_copy(a[:], av[hh][0:64, :])
                    nc.vector.tensor_copy(den2[0:1, hh * S_:(hh + 1) * S_],
                                          av[hh][64:65, :])
                    avs.append(a)
                recip2 = small_p.tile([1, 2 * S_], F32, tag="recip2")
                nc.vector.reciprocal(recip2[:], den2[:])
                recip2b = small_p.tile([1, 2 * S_], BF16, tag="recip2b")
                nc.vector.tensor_copy(recip2b[:], recip2[:])
                rb = ps_av.tile([128, S_], F32, tag="av", name=f"rb{j}")
                for hh in range(2):
                    nc.tensor.matmul(rb[hh * 64:(hh + 1) * 64, :],
                                     ones_row[:, 0:64],
                                     recip2b[0:1, hh * S_:(hh + 1) * S_],
                                     start=True, stop=True)
                ohp = oh_p.tile([128, S_], BF16, tag="oh")
                oh_sb.append(ohp)
                for hh in range(2):
                    # odd head writes the upper partition half (64-ch DVE
                    # ops may target either half)
                    nc.vector.tensor_mul(ohp[hh * 64:hh * 64 + 64, :],
                                         avs[hh][:],
                                         rb[hh * 64:hh * 64 + 64, :])

            # o-projection + residual (K=128 per head pair)
            for dt in range(Dt):
                wos = wo_p.tile([128, QT * 128], BF16, tag="wo")
                nc.gpsimd.dma_start(wos.rearrange("p (j c) -> p j c", c=128),
                                    wo[l, dt].rearrange("j p c -> p j c"))
                ps = ps_big.tile([128, 2 * S_], F32, tag="big")
                for jp in range(QT):
                    nc.tensor.matmul(ps[:, 0:S_],
                                     wos[:, jp * 128:(jp + 1) * 128],
                                     oh_sb[jp][:], start=(jp == 0),
                                     stop=(jp == QT - 1))
                nc.vector.tensor_add(x_sb[dt][:], x_sb[dt][:], ps[:, 0:S_])

            # ---------------- mlp sublayer ----------------
            h2 = rmsnorm_to_h(x_sb)
            oacc = []
            for ch in range(CH):
                prods = []
                for df in range(DFT):
                    w13s = w13_p.tile([128, 2 * Dt * 128], BF16, tag="w13")
                    nc.gpsimd.dma_start(
                        w13s.rearrange("p (u k c) -> p u k c", u=2, c=128),
                        w13[l, ch, df].rearrange("u k p c -> p u k c"))
                    gu = ps_big.tile([128, 2 * S_], F32, tag="big")
                    for kt in range(Dt):
                        nc.tensor.matmul(gu[:, 0:S_],
                                         w13s[:, kt * 128:(kt + 1) * 128],
                                         h2[kt][:], start=(kt == 0),
                                         stop=(kt == Dt - 1))
                    for kt in range(Dt):
                        off = Dt * 128
                        nc.tensor.matmul(
                            gu[:, S_:2 * S_],
                            w13s[:, off + kt * 128:off + (kt + 1) * 128],
                            h2[kt][:], start=(kt == 0), stop=(kt == Dt - 1))
                    gs = e_p.tile([128, 2 * S_], BF16, tag="e")
                    pr = prod_p.tile([128, S_], BF16, tag="prod")
                    if sim_safe:
                        # CoreSim lacks Silu; sigmoid + explicit mul
                        nc.scalar.activation(gs[:, 0:S_], gu[:, 0:S_],
                                             AF.Sigmoid)
                        gg = tmp_p.tile([128, S_], BF16, tag="gg")
                        nc.vector.tensor_mul(gg[:], gs[:, 0:S_], gu[:, 0:S_])
                        nc.vector.tensor_mul(pr[:], gg[:], gu[:, S_:2 * S_])
                    else:
                        nc.scalar.activation(gs[:, 0:S_], gu[:, 0:S_],
                                             AF.Silu)
                        nc.vector.tensor_mul(pr[:], gs[:, 0:S_],
                                             gu[:, S_:2 * S_])
                    prods.append(pr)
                for dt in range(Dt):
                    w2s = w2_p.tile([128, DFT * 128], BF16, tag="w2")
                    nc.gpsimd.dma_start(
                        w2s.rearrange("p (j c) -> p j c", c=128),
                        w2[l, ch, dt].rearrange("j p c -> p j c"))
                    ps = ps_big.tile([128, 2 * S_], F32, tag="big")
                    for jj in range(DFT):
                        nc.tensor.matmul(ps[:, 0:S_],
                                         w2s[:, jj * 128:(jj + 1) * 128],
                                         prods[jj][:], start=(jj == 0),
                                         stop=(jj == DFT - 1))
                    if ch == 0:
                        oa = oacc_p.tile([128, S_], F32, tag="oacc")
                        nc.vector.tensor_copy(oa[:], ps[:, 0:S_])
                        oacc.append(oa)
                    else:
                        nc.vector.tensor_add(oacc[dt][:], oacc[dt][:],
                                             ps[:, 0:S_])
            for dt in range(Dt):
                nc.vector.tensor_add(x_sb[dt][:], x_sb[dt][:], oacc[dt][:])

        # ---------------- final rmsnorm ----------------
        rn = rmsnorm_bcast(x_sb)
        for dt in range(Dt):
            xn = tmp_p.tile([128, S_], F32, tag="xn")
            nc.vector.tensor_mul(xn[:], x_sb[dt][:], rn[:])
            nc.vector.tensor_scalar_mul(xn[:], xn[:], fw_sb[:, dt:dt + 1])
            nc.sync.dma_start(out[dt * 128:(dt + 1) * 128, :], xn[:])

    nc.compile()
    return nc


# ---------------------------------------------------------------------------
# host-side preparation
# ---------------------------------------------------------------------------

def _bf16(a):
    return np.ascontiguousarray(np.asarray(a, dtype=np.float32)).astype(
        ml_dtypes.bfloat16)


def _perm(HD_):
    """Head-dim pair permutation: slot 2i <- dim i, slot 2i+1 <- dim i+HD/2."""
    half = HD_ // 2
    p = np.empty(HD_, dtype=np.int64)
    p[0::2] = np.arange(half)
    p[1::2] = np.arange(half) + half
    return p


def prepare_in_maps(x0, cos, sin, wq, wk, wv_, wo_, anw, mnw, w1, w3, w2_,
                    fnw, T_, S_, D_, NH_, HD_, DFF_, L_, n_cores=8):
    """Build per-core input dicts. x0 is the already-expanded [B', T, D] fp32
    input (B' = n_cores // G batches)."""
    Dt = D_ // 128
    QT = (NH_ * HD_) // 128
    KT = T_ // 128
    KP = KT // 2
    CH = max(1, DFF_ // 1024)
    DFT = (DFF_ // CH) // 128
    perm = _perm(HD_)

    # fold norm weights into the consuming projections
    wq_e = anw[:, :, None] * wq      # [L, D, D]
    wk_e = anw[:, :, None] * wk
    wv_e = anw[:, :, None] * wv_
    w1_e = mnw[:, :, None] * w1      # [L, D, DFF]
    w3_e = mnw[:, :, None] * w3

    # permute q/k columns per head by `perm`
    def permute_cols(w):
        wh = w.reshape(L_, D_, NH_, HD_)
        return wh[:, :, :, perm].reshape(L_, D_, NH_ * HD_)

    wq_p = permute_cols(wq_e)
    wk_p = permute_cols(wk_e)

    # packed q|k lhsT tiles: [L, 2QT, Dt, 128, 128]
    wqk_pack = np.empty((L_, 2 * QT, Dt, 128, 128), dtype=np.float32)
    for j in range(QT):
        for kt in range(Dt):
            wqk_pack[:, j, kt] = wq_p[:, kt * 128:(kt + 1) * 128,
                                      j * 128:(j + 1) * 128]
            wqk_pack[:, QT + j, kt] = wk_p[:, kt * 128:(kt + 1) * 128,
                                           j * 128:(j + 1) * 128]
    # wo packed: [L, Dt, QT, 128, 128]; rows = o dims (head-major)
    wo_pack = np.empty((L_, Dt, QT, 128, 128), dtype=np.float32)
    for dt in range(Dt):
        for j in range(QT):
            wo_pack[:, dt, j] = wo_[:, j * 128:(j + 1) * 128,
                                    dt * 128:(dt + 1) * 128]
    # w13 packed: [L, CH, DFT, 2, Dt, 128, 128]
    csz = DFF_ // CH
    w13_pack = np.empty((L_, CH, DFT, 2, Dt, 128, 128), dtype=np.float32)
    for ch in range(CH):
        for df in range(DFT):
            c0 = ch * csz + df * 128
            for kt in range(Dt):
                w13_pack[:, ch, df, 0, kt] = w1_e[:, kt * 128:(kt + 1) * 128,
                                                  c0:c0 + 128]
                w13_pack[:, ch, df, 1, kt] = w3_e[:, kt * 128:(kt + 1) * 128,
                                                  c0:c0 + 128]
    # w2 packed: [L, CH, Dt, DFT, 128, 128]
    w2_pack = np.empty((L_, CH, Dt, DFT, 128, 128), dtype=np.float32)
    for ch in range(CH):
        for dt in range(Dt):
            for j in range(DFT):
                r0 = ch * csz + j * 128
                w2_pack[:, ch, dt, j] = w2_[:, r0:r0 + 128,
                                            dt * 128:(dt + 1) * 128]

    wqk_b = _bf16(wqk_pack)
    wv_b = _bf16(wv_e)
    wo_b = _bf16(wo_pack)
    w13_b = _bf16(w13_pack)
    w2_b = _bf16(w2_pack)
    fw_np = np.ascontiguousarray(
        np.asarray(fnw, np.float32).reshape(Dt, 128).T)  # [128, Dt]

    # rope tables, permuted + sign-baked, duplicated per head pair -> [128, T]
    cosPf = np.asarray(cos, np.float32)[:, perm].T        # [HD, T]
    sinf = np.asarray(sin, np.float32)[:, perm].T         # [HD, T]
    sign = np.where(np.arange(HD_) % 2 == 0, -1.0, 1.0)[:, None]
    sinPf = sinf * sign
    cosP2 = np.tile(cosPf, (2, 1))                        # [128, T]
    sinP2 = np.tile(sinPf, (2, 1))

    in_maps = []
    for c in range(n_cores):
        b = c // G
        r = c % G
        t0 = r * S_
        xs = np.ascontiguousarray(x0[b, t0:t0 + S_, :].T).astype(np.float32)
        mask = np.zeros((KT, 128, S_), dtype=np.float32)
        for s in range(KT):
            kg = 128 * s + np.arange(128)[:, None]
            qg = t0 + np.arange(S_)[None, :]
            mask[s] = (kg <= qg).astype(np.float32)
        mask2 = mask.reshape(KP, 2, 128, S_).transpose(0, 2, 1, 3).reshape(
            KP, 128, 2 * S_)
        in_maps.append({
            "x0T": xs,
            "wqk": wqk_b, "wv": wv_b, "wo": wo_b, "w13": w13_b, "w2": w2_b,
            "cosP": _bf16(cosP2[:, t0:t0 + S_]),
            "sinP": _bf16(sinP2[:, t0:t0 + S_]),
            "masks": mask2.astype(ml_dtypes.bfloat16),
            "fw": fw_np,
        })
    return in_maps


def expand_input(x_processed, boundaries, counts, x_residual):
    """Ragged chunk expansion: token t of batch b takes chunk
    #{boundaries[b] <= t}, plus residual."""
    xp = np.asarray(x_processed, np.float32)
    bd = np.asarray(boundaries)
    xr = np.asarray(x_residual, np.float32)
    Bn, Tn, Dn = xr.shape
    tt = np.arange(Tn)
    out = np.empty_like(xr)
    for b in range(Bn):
        idx = np.searchsorted(bd[b], tt, side="right")
        out[b] = xp[b, idx, :] + xr[b]
    return out


_NC_CACHE = {}


def _get_nc(key):
    if key not in _NC_CACHE:
        _NC_CACHE[key] = build_decoder(*key)
    return _NC_CACHE[key]


def kernel(x_processed, boundaries, counts, x_residual, cos, sin, seq_len,
           wq, wk, wv, wo, attn_norm_w, mlp_norm_w, w1, w3, w2, final_norm_w,
           _trace=False):
    S_ = T // G
    x0 = expand_input(x_processed, boundaries, counts, x_residual)
    in_maps = prepare_in_maps(
        x0, cos, sin,
        np.asarray(wq, np.float32), np.asarray(wk, np.float32),
        np.asarray(wv, np.float32), np.asarray(wo, np.float32),
        np.asarray(attn_norm_w, np.float32), np.asarray(mlp_norm_w, np.float32),
        np.asarray(w1, np.float32), np.asarray(w3, np.float32),
        np.asarray(w2, np.float32), np.asarray(final_norm_w, np.float32),
        T, S_, D, NH, HD, DFF, L, n_cores=8)
    nc = _get_nc((T, S_, D, NH, HD, DFF, L, 8))
    res = run_bass_kernel_spmd(nc, in_maps, list(range(8)), trace=_trace)
    outp = np.empty((B, T, D), dtype=np.float32)
    for c in range(8):
        b, r = c // G, c % G
        outp[b, r * S_:(r + 1) * S_, :] = res.results[c]["out"].T
    if _trace:
        kernel.last_exec_time_ns = res.exec_time_ns
        kernel.last_results = res
    return outp


# revision 21
# speedup vs baseline: 1.1499x; 1.0456x over previous
"""Trainium2 Bass kernel for nn_Decoder_10230612099842.

2-layer decoder (rmsnorm / qkv+RoPE / causal attention / o-proj / rmsnorm /
silu-gated MLP / final rmsnorm) over a ragged-chunk-expanded input.

Strategy (8 NeuronCores = one TRN2 chip):
  - Host: ragged chunk expansion (searchsorted gather) + residual add, weight
    norm-folding, bf16 cast, head-dim pair-permutation for RoPE, per-core
    slicing.
  - Device: data-parallel over batch (2 groups of 4 cores), sequence-parallel
    over tokens within a group (512 tokens/core). Activations live
    feature-major [D, S].  Per layer, one bf16 AllGather of k and one of v
    inside each 4-core group; attention runs a uniform slot loop per head
    with per-core causal mask inputs (mask applied multiplicatively to
    exp-scores; a ones-column appended to v makes masked slots drop out of
    both the softmax numerator and denominator).  Scores for two k-slots
    share one 2-bank PSUM tile so exp/mask run as single wide ops, and the
    score matmuls are emitted one slot-pair ahead of the AV accumulation so
    the in-order PE never waits on the exp round-trip.
  - Matmuls in bf16 with fp32 PSUM accumulation; residual stream fp32.
"""

import numpy as np
import ml_dtypes
from contextlib import ExitStack

import concourse.bass as bass
import concourse.tile as tile
from concourse import bacc, mybir
from concourse.bass_utils import run_bass_kernel_spmd

F32 = mybir.dt.float32
BF16 = mybir.dt.bfloat16
AF = mybir.ActivationFunctionType

# model constants (full problem)
B, K, T, D, NH, HD, DFF, L = 2, 512, 2048, 1024, 16, 64, 4096, 2
EPS = 1e-5
G = 4  # cores per batch group

# stream_shuffle mask: swap adjacent partition pairs within each 32-quadrant
PAIR_MASK = [i ^ 1 for i in range(32)]


def build_decoder(T_, S_, D_, NH_, HD_, DFF_, L_, n_cores=8, sim_safe=False):
    """Build the SPMD decoder graph. S_ = tokens per core, T_ = total tokens
    per batch. Group size G divides cores into batch groups."""
    Dt = D_ // 128          # D partition-tiles
    QT = (NH_ * HD_) // 128  # head-pair tiles (2 heads per tile)
    KT = T_ // 128          # k-slots per head
    KP = KT // 2            # k-slot pairs
    TT = S_ // 128          # local token tiles
    CH = max(1, DFF_ // 1024)   # mlp chunks
    DFT = (DFF_ // CH) // 128   # dff tiles per chunk (8)
    WV = min(512, NH_ * HD_)    # v-proj psum width
    VH = (NH_ * HD_) // WV      # v-proj col halves
    HPV = WV // HD_             # heads per v-psum
    scale = 1.0 / float(np.sqrt(HD_))
    n_groups = n_cores // G
    rg = [list(range(g * G, (g + 1) * G)) for g in range(n_groups)]

    nc = bacc.Bacc("TRN2", target_bir_lowering=False, debug=False,
                   num_devices=n_cores)

    # ---- I/O ----
    x0T = nc.dram_tensor("x0T", [D_, S_], F32, kind="ExternalInput")
    wqk = nc.dram_tensor("wqk", [L_, 2 * QT, Dt, 128, 128], BF16,
                         kind="ExternalInput")  # packed q|k lhsT tiles
    wv = nc.dram_tensor("wv", [L_, D_, D_], BF16, kind="ExternalInput")
    wo = nc.dram_tensor("wo", [L_, Dt, QT, 128, 128], BF16,
                        kind="ExternalInput")  # packed per-dt slabs
    w13 = nc.dram_tensor("w13", [L_, CH, DFT, 2, Dt, 128, 128], BF16,
                         kind="ExternalInput")
    w2 = nc.dram_tensor("w2", [L_, CH, Dt, DFT, 128, 128], BF16,
                        kind="ExternalInput")
    cosP = nc.dram_tensor("cosP", [128, S_], BF16, kind="ExternalInput")
    sinP = nc.dram_tensor("sinP", [128, S_], BF16, kind="ExternalInput")
    masks = nc.dram_tensor("masks", [KP, 128, 2 * S_], BF16,
                           kind="ExternalInput")  # slot-pair packed
    fw = nc.dram_tensor("fw", [128, Dt], F32, kind="ExternalInput")
    out = nc.dram_tensor("out", [D_, S_], F32, kind="ExternalOutput")

    with tile.TileContext(nc) as tc, ExitStack() as ctx:
        # ---- pools ----
        singles = ctx.enter_context(tc.tile_pool(name="singles", bufs=1))
        wqk_p = ctx.enter_context(tc.tile_pool(name="wqk", bufs=3))
        wv_p = ctx.enter_context(tc.tile_pool(name="wv", bufs=Dt))
        wo_p = ctx.enter_context(tc.tile_pool(name="wo", bufs=3))
        w13_p = ctx.enter_context(tc.tile_pool(name="w13", bufs=3))
        w2_p = ctx.enter_context(tc.tile_pool(name="w2", bufs=3))
        h_p = ctx.enter_context(tc.tile_pool(name="h", bufs=Dt))
        q_p = ctx.enter_context(tc.tile_pool(name="q", bufs=QT))
        vstg_p = ctx.enter_context(tc.tile_pool(name="vstg", bufs=TT))
        kslab_p = ctx.enter_context(tc.tile_pool(name="kslab", bufs=2))
        vslab_p = ctx.enter_context(tc.tile_pool(name="vslab", bufs=3))
        e_p = ctx.enter_context(tc.tile_pool(name="e", bufs=4))
        tmp_p = ctx.enter_context(tc.tile_pool(name="tmp", bufs=2))
        oh_p = ctx.enter_context(tc.tile_pool(name="oh", bufs=QT))
        avs_p = ctx.enter_context(tc.tile_pool(name="avs", bufs=4))
        prod_p = ctx.enter_context(tc.tile_pool(name="prod", bufs=DFT + 1))
        oacc_p = ctx.enter_context(tc.tile_pool(name="oacc", bufs=Dt))
        sq_p = ctx.enter_context(tc.tile_pool(name="sq", bufs=3))
        small_p = ctx.enter_context(tc.tile_pool(name="small", bufs=2))
        ps_big = ctx.enter_context(tc.tile_pool(name="ps_big", bufs=3,
                                                space="PSUM"))
        ps_av = ctx.enter_context(tc.tile_pool(name="ps_av", bufs=2,
                                               space="PSUM"))
        dram = ctx.enter_context(tc.tile_pool(name="dram", bufs=2,
                                              space="DRAM"))

        # ---- persistent constants ----
        ones_col = singles.tile([128, 1], BF16, tag="ones_col")
        nc.vector.memset(ones_col[:], 1.0)
        eps_sb = singles.tile([1, 1], F32, tag="eps")
        nc.vector.memset(eps_sb[:], EPS)
        ones_row = singles.tile([1, 128], BF16, tag="ones_row")
        nc.vector.memset(ones_row[:], 1.0)
        cos_sb = singles.tile([128, S_], BF16, tag="cos")
        nc.sync.dma_start(cos_sb[:], cosP[:])
        sin_sb = singles.tile([128, S_], BF16, tag="sin")
        nc.sync.dma_start(sin_sb[:], sinP[:])
        fw_sb = singles.tile([128, Dt], F32, tag="fw")
        nc.sync.dma_start(fw_sb[:], fw[:])
        mask_sb = []
        for p in range(KP):
            m = singles.tile([128, 2 * S_], BF16, tag=f"mask{p}")
            nc.sync.dma_start(m[:], masks[p])
            mask_sb.append(m)
        x_sb = []
        for t in range(Dt):
            xt = singles.tile([128, S_], F32, tag=f"x{t}")
            nc.sync.dma_start(xt[:], x0T[t * 128:(t + 1) * 128, :])
            x_sb.append(xt)

        def rmsnorm_bcast(x_tiles):
            """Return a PSUM tile [128, S] holding rstd broadcast to all
            partitions (1/sqrt(mean(x^2)+eps) per token column)."""
            ssum = ps_av.tile([128, S_], F32, tag="av")
            for t in range(Dt):
                sq = sq_p.tile([128, S_], BF16, tag="sq")
                nc.vector.tensor_mul(sq[:], x_tiles[t][:], x_tiles[t][:])
                nc.tensor.matmul(ssum[0:1, :], ones_col[:], sq[:],
                                 start=(t == 0), stop=(t == Dt - 1))
            rstd = small_p.tile([1, S_], F32, tag="rstd")
            nc.scalar.activation(rstd[:], ssum[0:1, :], AF.Sqrt,
                                 bias=eps_sb[:], scale=1.0 / D_)
            nc.vector.reciprocal(rstd[:], rstd[:])
            rstd_bf = small_p.tile([1, S_], BF16, tag="rstd_bf")
            nc.vector.tensor_copy(rstd_bf[:], rstd[:])
            rn = ps_av.tile([128, S_], F32, tag="av")
            nc.tensor.matmul(rn[:], ones_row[:], rstd_bf[:],
                             start=True, stop=True)
            return rn

        def rmsnorm_to_h(x_tiles):
            rn = rmsnorm_bcast(x_tiles)
            hs = []
            for t in range(Dt):
                ht = h_p.tile([128, S_], BF16, tag="h")
                nc.vector.tensor_mul(ht[:], x_tiles[t][:], rn[:])
                hs.append(ht)
            return hs

        def rope_from_psum(ps, dst):
            """dst(bf16) = ps*cos + shuffle(ps)*sin  (pair-permuted RoPE)."""
            qb = tmp_p.tile([128, S_], BF16, tag="qb")
            nc.vector.tensor_copy(qb[:], ps[:])
            shuf = tmp_p.tile([128, S_], BF16, tag="shuf")
            nc.vector.stream_shuffle(shuf[:], qb[:], PAIR_MASK)
            qc = tmp_p.tile([128, S_], BF16, tag="qc")
            nc.vector.tensor_mul(qc[:], qb[:], cos_sb[:])
            nc.vector.tensor_mul(shuf[:], shuf[:], sin_sb[:])
            nc.vector.tensor_add(dst[:], qc[:], shuf[:])

        for l in range(L_):
            # ---------------- attention sublayer ----------------
            h = rmsnorm_to_h(x_sb)

            # k tiles first -> rope -> stage -> AllGather per head-half so
            # attention on the first heads can start while the rest is in
            # flight
            NHH = NH_ // 2
            QH = QT // 2
            kin_k = dram.tile([NH_, HD_, S_], BF16, tag="kin_k")
            kout_ks = []
            for ck in range(2):
                for j in range(ck * QH, (ck + 1) * QH):
                    wj = wqk_p.tile([128, Dt * 128], BF16, tag="wqk")
                    nc.gpsimd.dma_start(
                        wj.rearrange("p (k c) -> p k c", c=128),
                        wqk[l, QT + j].rearrange("k p c -> p k c"))
                    ps = ps_big.tile([128, 2 * S_], F32, tag="big")
                    for kt in range(Dt):
                        nc.tensor.matmul(ps[:, 0:S_],
                                         wj[:, kt * 128:(kt + 1) * 128],
                                         h[kt][:], start=(kt == 0),
                                         stop=(kt == Dt - 1))
                    kj = tmp_p.tile([128, S_], BF16, tag="kj")
                    rope_from_psum(ps[:, 0:S_], kj)
                    nc.sync.dma_start(
                        kin_k[2 * j:2 * j + 2].rearrange("h d s -> (h d) s"),
                        kj[:])
                ko = dram.tile([G, NHH, HD_, S_], BF16, tag=f"kout_k{ck}")
                nc.gpsimd.collective_compute(
                    "AllGather", mybir.AluOpType.bypass, replica_groups=rg,
                    ins=[kin_k[ck * NHH:(ck + 1) * NHH].opt()],
                    outs=[ko.opt()])
                kout_ks.append(ko)

            # v tiles (token-major, no ones column: the receiver-side slab
            # carries a 64-wide ones block instead) -> AllGather per half
            wv_sb = []
            for kt in range(Dt):
                wt = wv_p.tile([128, D_], BF16, tag="wv")
                nc.gpsimd.dma_start(wt[:], wv[l, kt * 128:(kt + 1) * 128, :])
                wv_sb.append(wt)
            kin_v = dram.tile([NH_, S_, HD_], BF16, tag="kin_v")
            kout_vs = []
            for half in range(VH):
                for tt in range(TT):
                    ps = ps_big.tile([128, 2 * S_], F32, tag="big")
                    for kt in range(Dt):
                        nc.tensor.matmul(
                            ps[:, 0:WV], h[kt][:, tt * 128:(tt + 1) * 128],
                            wv_sb[kt][:, half * WV:(half + 1) * WV],
                            start=(kt == 0), stop=(kt == Dt - 1))
                    vs = vstg_p.tile([128, WV], BF16, tag="vstg")
                    nc.vector.tensor_copy(vs[:], ps[:, 0:WV])
                    nc.sync.dma_start(
                        kin_v[half * HPV:(half + 1) * HPV].rearrange(
                            "h s c -> s h c")[tt * 128:(tt + 1) * 128],
                        vs.rearrange("p (h c) -> p h c", c=HD_))
                vo = dram.tile([G, HPV, S_, HD_], BF16, tag=f"kout_v{half}")
                nc.gpsimd.collective_compute(
                    "AllGather", mybir.AluOpType.bypass, replica_groups=rg,
                    ins=[kin_v[half * HPV:(half + 1) * HPV].opt()],
                    outs=[vo.opt()])
                kout_vs.append(vo)

            # q tiles (feature-major, rope'd) — overlaps the AllGathers
            q_sb = []
            for j in range(QT):
                wj = wqk_p.tile([128, Dt * 128], BF16, tag="wqk")
                nc.gpsimd.dma_start(wj.rearrange("p (k c) -> p k c", c=128),
                                    wqk[l, j].rearrange("k p c -> p k c"))
                ps = ps_big.tile([128, 2 * S_], F32, tag="big")
                for kt in range(Dt):
                    nc.tensor.matmul(ps[:, 0:S_],
                                     wj[:, kt * 128:(kt + 1) * 128],
                                     h[kt][:], start=(kt == 0),
                                     stop=(kt == Dt - 1))
                qj = q_p.tile([128, S_], BF16, tag="q")
                rope_from_psum(ps[:, 0:S_], qj)
                q_sb.append(qj)

            # attention, one head PAIR at a time; two k-slots share one
            # 2-bank PSUM tile so exp/mask are single wide ops; score matmuls
            # run one slot-pair ahead of the AV accumulation
            oh_sb = []
            for j in range(QT):
                kpair = kslab_p.tile([128, KT * 128], BF16, tag="kslab")
                for hh in range(2):
                    head = 2 * j + hh
                    ko = kout_ks[head // NHH]
                    nc.sync.dma_start(
                        kpair[hh * 64:(hh + 1) * 64, :].rearrange(
                            "d (g s) -> d g s", g=G),
                        ko[:, head % NHH].rearrange("g d s -> d g s"))
                vsl = []
                for hh in range(2):
                    head = 2 * j + hh
                    vo = kout_vs[head // HPV]
                    # slot layout [v(64) | ones(64)]: AV rows 64-127 become
                    # the softmax denominator replicated across partitions
                    vslab = vslab_p.tile([128, KT, 128], BF16, tag="vslab",
                                         name=f"vslab{j}_{hh}")
                    for g in range(G):
                        nc.sync.dma_start(
                            vslab[:, g * TT:(g + 1) * TT, 0:HD_],
                            vo[g, head % HPV].rearrange(
                                "(t p) c -> p t c", p=128))
                    nc.gpsimd.memset(vslab[:, :, HD_:128], 1.0)
                    vsl.append(vslab)
                av = [ps_av.tile([128, S_], F32, tag="av", name=f"av{j}_{x}")
                      for x in range(2)]
                es_q = []
                for p in range(KP):
                    es = []
                    for hh in range(2):
                        sc2 = ps_big.tile([128, 2 * S_], F32, tag="big")
                        for u in range(2):
                            s = 2 * p + u
                            nc.tensor.matmul(
                                sc2[:, u * S_:(u + 1) * S_],
                                kpair[hh * 64:hh * 64 + 64,
                                      s * 128:(s + 1) * 128],
                                q_sb[j][hh * 64:hh * 64 + 64, :],
                                start=True, stop=True)
                        e2 = e_p.tile([128, 2 * S_], BF16, tag="e")
                        nc.scalar.activation(e2[:], sc2[:], AF.Exp,
                                             scale=scale)
                        nc.vector.tensor_mul(e2[:], e2[:], mask_sb[p][:])
                        es.append(e2)
                    es_q.append(es)
                    if p >= 1:
                        for hh in range(2):
                            for u in range(2):
                                s = 2 * (p - 1) + u
                                nc.tensor.matmul(
                                    av[hh][:], vsl[hh][:, s, :],
                                    es_q[p - 1][hh][:, u * S_:(u + 1) * S_],
                                    start=(s == 0), stop=(s == KT - 1))
                for hh in range(2):
                    for u in range(2):
                        s = 2 * (KP - 1) + u
                        nc.tensor.matmul(
                            av[hh][:], vsl[hh][:, s, :],
                            es_q[KP - 1][hh][:, u * S_:(u + 1) * S_],
                            start=(s == 0), stop=(s == KT - 1))
                # av rows 0-63 = numerator, rows 64-127 = denominator
                # replicated; normalize with a 64-lane reciprocal + mul
                ohp = oh_p.tile([128, S_], BF16, tag="oh")
                oh_sb.append(ohp)
                for hh in range(2):
                    rc = avs_p.tile([64, S_], F32, tag="rc",
                                    name=f"rc{j}_{hh}")
                    nc.vector.reciprocal(rc[:], av[hh][64:128, :])
                    # odd head writes the upper partition half (64-ch DVE
                    # ops may target either half)
                    nc.vector.tensor_mul(ohp[hh * 64:hh * 64 + 64, :],
                                         av[hh][0:64, :], rc[:])

            # o-projection + residual (K=128 per head pair)
            for dt in range(Dt):
                wos = wo_p.tile([128, QT * 128], BF16, tag="wo")
                nc.gpsimd.dma_start(wos.rearrange("p (j c) -> p j c", c=128),
                                    wo[l, dt].rearrange("j p c -> p j c"))
                ps = ps_big.tile([128, 2 * S_], F32, tag="big")
                for jp in range(QT):
                    nc.tensor.matmul(ps[:, 0:S_],
                                     wos[:, jp * 128:(jp + 1) * 128],
                                     oh_sb[jp][:], start=(jp == 0),
                                     stop=(jp == QT - 1))
                nc.vector.tensor_add(x_sb[dt][:], x_sb[dt][:], ps[:, 0:S_])

            # ---------------- mlp sublayer ----------------
            h2 = rmsnorm_to_h(x_sb)
            oacc = []
            for ch in range(CH):
                prods = []
                for df in range(DFT):
                    w13s = w13_p.tile([128, 2 * Dt * 128], BF16, tag="w13")
                    nc.gpsimd.dma_start(
                        w13s.rearrange("p (u k c) -> p u k c", u=2, c=128),
                        w13[l, ch, df].rearrange("u k p c -> p u k c"))
                    gu = ps_big.tile([128, 2 * S_], F32, tag="big")
                    for kt in range(Dt):
                        nc.tensor.matmul(gu[:, 0:S_],
                                         w13s[:, kt * 128:(kt + 1) * 128],
                                         h2[kt][:], start=(kt == 0),
                                         stop=(kt == Dt - 1))
                    for kt in range(Dt):
                        off = Dt * 128
                        nc.tensor.matmul(
                            gu[:, S_:2 * S_],
                            w13s[:, off + kt * 128:off + (kt + 1) * 128],
                            h2[kt][:], start=(kt == 0), stop=(kt == Dt - 1))
                    gs = e_p.tile([128, 2 * S_], BF16, tag="e")
                    pr = prod_p.tile([128, S_], BF16, tag="prod")
                    if sim_safe:
                        # CoreSim lacks Silu; sigmoid + explicit mul
                        nc.scalar.activation(gs[:, 0:S_], gu[:, 0:S_],
                                             AF.Sigmoid)
                        gg = tmp_p.tile([128, S_], BF16, tag="gg")
                        nc.vector.tensor_mul(gg[:], gs[:, 0:S_], gu[:, 0:S_])
                        nc.vector.tensor_mul(pr[:], gg[:], gu[:, S_:2 * S_])
                    else:
                        nc.scalar.activation(gs[:, 0:S_], gu[:, 0:S_],
                                             AF.Silu)
                        nc.vector.tensor_mul(pr[:], gs[:, 0:S_],
                                             gu[:, S_:2 * S_])
                    prods.append(pr)
                for dt in range(Dt):
                    w2s = w2_p.tile([128, DFT * 128], BF16, tag="w2")
                    nc.gpsimd.dma_start(
                        w2s.rearrange("p (j c) -> p j c", c=128),
                        w2[l, ch, dt].rearrange("j p c -> p j c"))
                    ps = ps_big.tile([128, 2 * S_], F32, tag="big")
                    for jj in range(DFT):
                        nc.tensor.matmul(ps[:, 0:S_],
                                         w2s[:, jj * 128:(jj + 1) * 128],
                                         prods[jj][:], start=(jj == 0),
                                         stop=(jj == DFT - 1))
                    if ch == 0:
                        oa = oacc_p.tile([128, S_], F32, tag="oacc")
                        nc.vector.tensor_copy(oa[:], ps[:, 0:S_])
                        oacc.append(oa)
                    else:
                        nc.vector.tensor_add(oacc[dt][:], oacc[dt][:],
                                             ps[:, 0:S_])
            for dt in range(Dt):
                nc.vector.tensor_add(x_sb[dt][:], x_sb[dt][:], oacc[dt][:])

        # ---------------- final rmsnorm ----------------
        rn = rmsnorm_bcast(x_sb)
        for dt in range(Dt):
            xn = tmp_p.tile([128, S_], F32, tag="xn")
            nc.vector.tensor_mul(xn[:], x_sb[dt][:], rn[:])
            nc.vector.tensor_scalar_mul(xn[:], xn[:], fw_sb[:, dt:dt + 1])
            nc.sync.dma_start(out[dt * 128:(dt + 1) * 128, :], xn[:])

    nc.compile()
    return nc


# ---------------------------------------------------------------------------
# host-side preparation
# ---------------------------------------------------------------------------

def _bf16(a):
    return np.ascontiguousarray(np.asarray(a, dtype=np.float32)).astype(
        ml_dtypes.bfloat16)


def _perm(HD_):
    """Head-dim pair permutation: slot 2i <- dim i, slot 2i+1 <- dim i+HD/2."""
    half = HD_ // 2
    p = np.empty(HD_, dtype=np.int64)
    p[0::2] = np.arange(half)
    p[1::2] = np.arange(half) + half
    return p


def prepare_in_maps(x0, cos, sin, wq, wk, wv_, wo_, anw, mnw, w1, w3, w2_,
                    fnw, T_, S_, D_, NH_, HD_, DFF_, L_, n_cores=8):
    """Build per-core input dicts. x0 is the already-expanded [B', T, D] fp32
    input (B' = n_cores // G batches)."""
    Dt = D_ // 128
    QT = (NH_ * HD_) // 128
    KT = T_ // 128
    KP = KT // 2
    CH = max(1, DFF_ // 1024)
    DFT = (DFF_ // CH) // 128
    perm = _perm(HD_)

    # fold norm weights into the consuming projections
    wq_e = anw[:, :, None] * wq      # [L, D, D]
    wk_e = anw[:, :, None] * wk
    wv_e = anw[:, :, None] * wv_
    w1_e = mnw[:, :, None] * w1      # [L, D, DFF]
    w3_e = mnw[:, :, None] * w3

    # permute q/k columns per head by `perm`
    def permute_cols(w):
        wh = w.reshape(L_, D_, NH_, HD_)
        return wh[:, :, :, perm].reshape(L_, D_, NH_ * HD_)

    wq_p = permute_cols(wq_e)
    wk_p = permute_cols(wk_e)

    # packed q|k lhsT tiles: [L, 2QT, Dt, 128, 128]
    wqk_pack = np.empty((L_, 2 * QT, Dt, 128, 128), dtype=np.float32)
    for j in range(QT):
        for kt in range(Dt):
            wqk_pack[:, j, kt] = wq_p[:, kt * 128:(kt + 1) * 128,
                                      j * 128:(j + 1) * 128]
            wqk_pack[:, QT + j, kt] = wk_p[:, kt * 128:(kt + 1) * 128,
                                           j * 128:(j + 1) * 128]
    # wo packed: [L, Dt, QT, 128, 128]; rows = o dims (head-major)
    wo_pack = np.empty((L_, Dt, QT, 128, 128), dtype=np.float32)
    for dt in range(Dt):
        for j in range(QT):
            wo_pack[:, dt, j] = wo_[:, j * 128:(j + 1) * 128,
                                    dt * 128:(dt + 1) * 128]
    # w13 packed: [L, CH, DFT, 2, Dt, 128, 128]
    csz = DFF_ // CH
    w13_pack = np.empty((L_, CH, DFT, 2, Dt, 128, 128), dtype=np.float32)
    for ch in range(CH):
        for df in range(DFT):
            c0 = ch * csz + df * 128
            for kt in range(Dt):
                w13_pack[:, ch, df, 0, kt] = w1_e[:, kt * 128:(kt + 1) * 128,
                                                  c0:c0 + 128]
                w13_pack[:, ch, df, 1, kt] = w3_e[:, kt * 128:(kt + 1) * 128,
                                                  c0:c0 + 128]
    # w2 packed: [L, CH, Dt, DFT, 128, 128]
    w2_pack = np.empty((L_, CH, Dt, DFT, 128, 128), dtype=np.float32)
    for ch in range(CH):
        for dt in range(Dt):
            for j in range(DFT):
                r0 = ch * csz + j * 128
                w2_pack[:, ch, dt, j] = w2_[:, r0:r0 + 128,
                                            dt * 128:(dt + 1) * 128]

    wqk_b = _bf16(wqk_pack)
    wv_b = _bf16(wv_e)
    wo_b = _bf16(wo_pack)
    w13_b = _bf16(w13_pack)
    w2_b = _bf16(w2_pack)
    fw_np = np.ascontiguousarray(
        np.asarray(fnw, np.float32).reshape(Dt, 128).T)  # [128, Dt]

    # rope tables, permuted + sign-baked, duplicated per head pair -> [128, T]
    cosPf = np.asarray(cos, np.float32)[:, perm].T        # [HD, T]
    sinf = np.asarray(sin, np.float32)[:, perm].T         # [HD, T]
    sign = np.where(np.arange(HD_) % 2 == 0, -1.0, 1.0)[:, None]
    sinPf = sinf * sign
    cosP2 = np.tile(cosPf, (2, 1))                        # [128, T]
    sinP2 = np.tile(sinPf, (2, 1))

    in_maps = []
    for c in range(n_cores):
        b = c // G
        r = c % G
        t0 = r * S_
        xs = np.ascontiguousarray(x0[b, t0:t0 + S_, :].T).astype(np.float32)
        mask = np.zeros((KT, 128, S_), dtype=np.float32)
        for s in range(KT):
            kg = 128 * s + np.arange(128)[:, None]
            qg = t0 + np.arange(S_)[None, :]
            mask[s] = (kg <= qg).astype(np.float32)
        mask2 = mask.reshape(KP, 2, 128, S_).transpose(0, 2, 1, 3).reshape(
            KP, 128, 2 * S_)
        in_maps.append({
            "x0T": xs,
            "wqk": wqk_b, "wv": wv_b, "wo": wo_b, "w13": w13_b, "w2": w2_b,
            "cosP": _bf16(cosP2[:, t0:t0 + S_]),
            "sinP": _bf16(sinP2[:, t0:t0 + S_]),
            "masks": mask2.astype(ml_dtypes.bfloat16),
            "fw": fw_np,
        })
    return in_maps


def expand_input(x_processed, boundaries, counts, x_residual):
    """Ragged chunk expansion: token t of batch b takes chunk
    #{boundaries[b] <= t}, plus residual."""
    xp = np.asarray(x_processed, np.float32)
    bd = np.asarray(boundaries)
    xr = np.asarray(x_residual, np.float32)
    Bn, Tn, Dn = xr.shape
    tt = np.arange(Tn)
    out = np.empty_like(xr)
    for b in range(Bn):
        idx = np.searchsorted(bd[b], tt, side="right")
        out[b] = xp[b, idx, :] + xr[b]
    return out


_NC_CACHE = {}


def _get_nc(key):
    if key not in _NC_CACHE:
        _NC_CACHE[key] = build_decoder(*key)
    return _NC_CACHE[key]


def kernel(x_processed, boundaries, counts, x_residual, cos, sin, seq_len,
           wq, wk, wv, wo, attn_norm_w, mlp_norm_w, w1, w3, w2, final_norm_w,
           _trace=False):
    S_ = T // G
    x0 = expand_input(x_processed, boundaries, counts, x_residual)
    in_maps = prepare_in_maps(
        x0, cos, sin,
        np.asarray(wq, np.float32), np.asarray(wk, np.float32),
        np.asarray(wv, np.float32), np.asarray(wo, np.float32),
        np.asarray(attn_norm_w, np.float32), np.asarray(mlp_norm_w, np.float32),
        np.asarray(w1, np.float32), np.asarray(w3, np.float32),
        np.asarray(w2, np.float32), np.asarray(final_norm_w, np.float32),
        T, S_, D, NH, HD, DFF, L, n_cores=8)
    nc = _get_nc((T, S_, D, NH, HD, DFF, L, 8))
    res = run_bass_kernel_spmd(nc, in_maps, list(range(8)), trace=_trace)
    outp = np.empty((B, T, D), dtype=np.float32)
    for c in range(8):
        b, r = c // G, c % G
        outp[b, r * S_:(r + 1) * S_, :] = res.results[c]["out"].T
    if _trace:
        kernel.last_exec_time_ns = res.exec_time_ns
        kernel.last_results = res
    return outp
